# revision 9
# baseline (speedup 1.0000x reference)
"""Trainium2 Bass kernel for ContinuousWaveletLayer (CWT energy), v3.

Reference computation:
  bank = Morlet wavelet bank [32 scales, Lmax=256] (static)
  coef[b,s,t] = 'same' conv of x[b,:] (len 8192) with bank[s,:]
  out[b,s]    = mean_t(coef^2) * softmax(scale_weights)[s]

v3 strategy (vs v2's 54.5us):
  * Morlet coefficients at scale s are band-limited (center 5/s rad,
    Gaussian width ~1/s), so mean_t(coef^2) can be estimated from a
    stride-4 time subsample (x4) for s >= 9 with < 1e-3 aliasing error
    (validated numerically; s=9 is 2.5e-3).  This cuts a scale's PE
    cost from 8192/16384 streamed columns to 6144 and its square /
    reduce cost 4x.
  * Uniform SPMD shape: every core runs 1 exact small scale (1..8) +
    3 strided scales; all 65 small-scale out-blocks are offset by +64
    so a single K=256 DoubleRow window covers the kernel support with
    UNSHIFTED x (edge out-blocks use partial-partition squares instead
    of masked weights); the +64-shifted x copy of v2 is gone, halving
    input DMA to 1.38MB/core.
  * Strided conv matmuls use 4-level rhs APs [p][j][m'][b] so one
    N=512 matmul covers 4 decimated out-blocks.
  * fp8(e4m3) everywhere; DR reduces over fp8 squares; deterministic
    fp8 norm bias divided out exactly on the host (as in v2).
"""

import sys
from contextlib import ExitStack

import numpy as np

sys.path.insert(0, "/opt/trn_rl_repo")

import concourse.bass as bass
import concourse.mybir as mybir
from concourse import tile
from concourse.bass_utils import run_bass_kernel_spmd
from concourse.vector_clock import ScopedClock


def _drain_and_barrier_single_wait(self, tick_clock, wait_clock):
    """TileContext._drain_and_barrier, but the kernel-tail drain's
    global-clock waits are spread over a chain of single-wait drains —
    the walrus build here allows only one sync wait per instruction."""
    drain_inst = self.nc.sync.drain()
    wait_clock.add_sem_waits(
        drain_inst.ins, ScopedClock({None: tick_clock.global_clock})
    )
    si = drain_inst.ins.sync_info
    waits = list(si.on_wait)
    if len(waits) > 1:
        si.on_wait = [waits[0]]
        sems = {h.name: h for h in self.sems.allocated().values()}
        for w in waits[1:]:
            d2 = self.nc.sync.drain()
            d2.wait_op(sems[w.ant_name], w.wait_value, "sem-ge")
    self.nc.all_engine_barrier()
    assert self.sems is not None
    popped = self.nc._tile_sem_poison_stack.pop()
    assert popped is self._sem_poison
    self.nc.clear_and_free_semaphores(list(self.sems.allocated().values()))
    self.nc.all_engine_barrier()


tile.TileContext._drain_and_barrier = _drain_and_barrier_single_wait

N_CORES = 8
S_TOTAL = 32
P = 128
NT = 8192
LMAX = 256
NBLK = 66             # x blocks incl. 1 pad block each side
F32 = mybir.dt.float32
BF16 = mybir.dt.bfloat16
FP8 = mybir.dt.float8e4
DR = mybir.MatmulPerfMode.DoubleRow

# weights region: small-scale lhsT (256 cols) + 3 strided scales (3x256)
W_NS = 0                      # small-scale lhsT [128, (j2,128)]
W_S4 = [256, 1024, 1792]      # strided lhsT base (3 passes x 256 each)
WCOL = 2560
XB = WCOL                     # x region base col
NCOL = WCOL + NBLK * P        # 11008

# input DMA chunks (col ranges, queue): consumption-ordered; the two
# early x chunks ride the ACT hwdge queue so their transfers overlap the
# weights chunks on the SP queue.
CHUNKS = [
    (0, 256, "sp"),                            # small-scale lhsT
    (XB, XB + 10 * P, "act"),                  # x blocks 0..9
    (256, WCOL, "sp"),                         # strided lhsT
    (XB + 10 * P, XB + 19 * P, "act"),         # x blocks 10..18
    (XB + 19 * P, XB + 35 * P, "sp"),          # x blocks 19..34
    (XB + 35 * P, XB + NBLK * P, "sp"),        # x blocks 35..65
]
# issue order: alternate queues so both queues' first transfers begin
# as early as possible
ISSUE_ORDER = [0, 1, 2, 3, 4, 5]

# square-engine map: True = ACT (1-op square), False = DVE (copy+mul).
# Regions are 1024-col DR reduce pairs; both halves of a pair must be
# written by the SAME engine (single sync wait on the reduce).
NS_PAIR_ACT = [False, True, True, False, True, True, True, True]  # r0..r7
NS_TAIL_ACT = False
S4_PAIR_ACT = [[False, True], [True, True], [False, True]]  # [scale][pair]

LAST_RESULTS = None


def _morlet_kernel_bank(n_scales: int, n: int) -> np.ndarray:
    Lmax = min(8 * n_scales, n)
    bank = np.zeros((n_scales, Lmax), dtype=np.float32)
    for i, s in enumerate(range(1, n_scales + 1)):
        L = min(8 * s, n)
        t = np.linspace(-4.0 * s, 4.0 * s, L)
        w = np.exp(-t**2 / (2.0 * s**2)) * np.cos(5.0 * t / s)
        w = w / np.sqrt(s)
        off = (Lmax - 1) // 2 - (L - 1) // 2
        bank[i, off : off + L] = w.astype(np.float32)
    return bank


def _core_scales(c: int) -> list[int]:
    """0-based scale ids on core c: [small, s4a, s4b, s4c]."""
    return [c, 8 + 3 * c, 9 + 3 * c, 10 + 3 * c]


def _gslice(row, idx):
    v = np.zeros(idx.shape, dtype=np.float32)
    ok = (idx >= 0) & (idx < LMAX)
    v[ok] = row[np.clip(idx, 0, LMAX - 1)][ok]
    return v


def _lhsT_small(gq: np.ndarray) -> np.ndarray:
    """[128, 256] cols (j, to): w[p, 128j+to] = g'[128j + p - to + 63]."""
    p = np.arange(P)[:, None]
    to = np.arange(P)[None, :]
    return np.concatenate(
        [_gslice(gq, 128 * j + p - to + 63) for j in (0, 1)], axis=1
    )


def _lhsT_s4(gq: np.ndarray, q: int) -> np.ndarray:
    """stride-4 pass q: w[p, 128j+to] = g'[256q + 128j + p - 4to - 4]."""
    p = np.arange(P)[:, None]
    to = np.arange(P)[None, :]
    return np.concatenate(
        [_gslice(gq, 256 * q + 128 * j + p - 4 * to - 4) for j in (0, 1)],
        axis=1,
    )


def _xw4(xgsb, base_col, mstride):
    """4-level rhs AP [p][j:128,2][m:mstride,4][b:1,128] at base_col."""
    sl = xgsb[:, base_col : base_col + P]
    return bass.AP(
        sl.tensor, sl.offset, [list(sl.ap[0]), [P, 2], [mstride, 4], [1, P]]
    )


def _xw3(xgsb, base_col, n):
    """3-level rhs AP [p][j:128,2][n:1,n] at base_col."""
    sl = xgsb[:, base_col : base_col + P]
    return bass.AP(sl.tensor, sl.offset, [list(sl.ap[0]), [P, 2], [1, n]])


def _pair_ap(sq, lo, jstride, n):
    """DR reduce rhs [p][j:jstride,2][n:1,n] over sq fp8 tile at col lo."""
    sl = sq[:, lo : lo + n]
    return bass.AP(sl.tensor, sl.offset, [list(sl.ap[0]), [jstride, 2], [1, n]])


def _build_nc() -> bass.Bass:
    nc = bass.Bass()
    xg = nc.dram_tensor("xg", [P, NCOL], FP8, kind="ExternalInput")
    outp = nc.dram_tensor("outp", [1, 4 * 512], F32, kind="ExternalOutput")

    with tile.TileContext(nc) as tc, ExitStack() as ctx:
        xpool = ctx.enter_context(tc.tile_pool(name="x", bufs=1))
        sqpool = ctx.enter_context(tc.tile_pool(name="sq", bufs=1))
        cppool = ctx.enter_context(tc.tile_pool(name="cp", bufs=4))
        rowpool = ctx.enter_context(tc.tile_pool(name="row", bufs=1))
        pspool = ctx.enter_context(tc.tile_pool(name="ps", bufs=4, space="PSUM"))
        psepool = ctx.enter_context(tc.tile_pool(name="pse", bufs=1, space="PSUM"))

        xgsb = xpool.tile([P, NCOL], FP8)
        scr = xpool.tile([P, 512], FP8, name="scr")     # warmup scratch
        ones = xpool.tile([P, 32], FP8, name="ones")
        sqNS = sqpool.tile([P, 8320], FP8, name="sqNS")
        sqS4 = [sqpool.tile([P, 2048], FP8, name=f"sqS4_{k}") for k in range(3)]
        rowout = rowpool.tile([1, 4 * 512], F32, name="rowout")

        # input DMA chunks on two hwdge queues
        for ci in ISSUE_ORDER:
            lo, hi, q = CHUNKS[ci]
            eng = nc.sync if q == "sp" else nc.scalar
            eng.dma_start(out=xgsb[:, lo:hi], in_=xg[:, lo:hi])

        # warmup scratch on the otherwise-idle GpSimd engine so the PE
        # warmups start right after the preamble, gated by nothing else
        nc.gpsimd.memset(scr[:, :], 1.0)
        # DVE init: ones + sq edge zeros (the small-scale edge out-blocks
        # write only half their partitions; the other half must read 0 in
        # the reduce)
        nc.vector.memset(ones[:, :], 1.0)
        nc.vector.memset(sqNS[0:64, 0:128], 0.0)
        nc.vector.memset(sqNS[64:128, 8192:8320], 0.0)

        # acc bank: rows 0 of four 512-col regions hold the per-scale
        # energy accumulators; warmup/guard matmuls write the (otherwise
        # unused) full-partition region before any reduce starts
        accbank = psepool.tile([P, 4 * 512], F32, name="accbank")

        # warmup matmuls on (uninitialized) scratch: ramp the PE clock
        # while input DMA is still in flight (no dependency at all; the
        # garbage results are overwritten when the reduces start)
        for _ in range(4):
            nc.tensor.matmul(
                accbank[:, 0:512], scr[:, 0:128], scr[:, :], start=True, stop=True,
                skip_group_check=True,
            )

        # guard matmuls: one per input chunk; each carries that chunk's DMA
        # sem wait so real matmuls below never need a second wait
        def guard(chunk_idx):
            # writes partitions 64.. only: partition 0 holds the live
            # energy accumulators, which guards must not reset
            lo = CHUNKS[chunk_idx][0]
            nc.tensor.matmul(
                accbank[64:128, 0:64], xgsb[:, lo : lo + 64], xgsb[:, lo : lo + 64],
                start=True, stop=True, skip_group_check=True,
            )

        # per-scale PSUM energy accumulator views [1, 512]
        accs = [accbank[0:1, 512 * i : 512 * i + 512] for i in range(4)]
        acc_started = [False] * 4
        acc_nred = [9, 2, 2, 2]      # reduces per acc
        acc_done = [0] * 4

        lhsT_NS = xgsb[:, W_NS : W_NS + 256].rearrange("p (j t) -> p j t", j=2)
        lhsT_S4 = [
            [
                xgsb[:, W_S4[k] + 256 * q : W_S4[k] + 256 * q + 256].rearrange(
                    "p (j t) -> p j t", j=2
                )
                for q in range(3)
            ]
            for k in range(3)
        ]
        ones_dr = bass.AP(
            ones.tensor, ones[:, :].offset, [list(ones[:, :].ap[0]), [16, 2], [1, 1]]
        )

        def square(eng_act, dst_sq, lo, n, pt, plo=0, phi=P):
            """square psum [plo:phi, 0:n] into dst_sq[plo:phi, lo:lo+n]."""
            if eng_act:
                nc.scalar.square(dst_sq[plo:phi, lo : lo + n], pt[plo:phi, 0:n])
            else:
                cp = cppool.tile([P, 512], BF16)
                nc.vector.tensor_copy(cp[plo:phi, 0:n], pt[plo:phi, 0:n])
                nc.vector.tensor_mul(
                    dst_sq[plo:phi, lo : lo + n], cp[plo:phi, 0:n], cp[plo:phi, 0:n]
                )

        def reduce(ai, rhs_ap, n, dr=True):
            acc_done[ai] += 1
            nc.tensor.matmul(
                accs[ai][:, 0:n], ones_dr if dr else ones[:, 0:1], rhs_ap,
                start=not acc_started[ai], stop=acc_done[ai] == acc_nred[ai],
                perf_mode=DR if dr else None, skip_group_check=True,
            )
            acc_started[ai] = True

        # ---- helpers for schedule ----
        def conv_NS(g):
            pt = pspool.tile([P, 512], F32, tag="conv")
            nc.tensor.matmul(
                pt[:, :], lhsT_NS, _xw4(xgsb, XB + 512 * g, P),
                start=True, stop=True, perf_mode=DR,
            )
            act = NS_PAIR_ACT[g // 2]
            if g == 0:
                # m=-1 edge: only out-times 0..63 (partitions 64:) valid
                square(act, sqNS, 0, 128, pt, plo=64)
                sl = sqNS[:, 128:512]
                if act:
                    nc.scalar.square(sl, pt[:, 128:512])
                else:
                    cp = cppool.tile([P, 512], BF16)
                    nc.vector.tensor_copy(cp[:, 0:384], pt[:, 128:512])
                    nc.vector.tensor_mul(sl, cp[:, 0:384], cp[:, 0:384])
            else:
                square(act, sqNS, 512 * g, 512, pt)

        def conv_S4(k, T):
            pt = pspool.tile([P, 512], F32, tag="conv")
            for q in range(3):
                nc.tensor.matmul(
                    pt[:, :], lhsT_S4[k][q],
                    _xw4(xgsb, XB + P * (16 * T + 2 * q), 4 * P),
                    start=q == 0, stop=q == 2, perf_mode=DR,
                )
            square(S4_PAIR_ACT[k][T // 2], sqS4[k], 512 * T, 512, pt)

        def conv_NS_tail():
            pt = pspool.tile([P, 512], F32, tag="conv")
            nc.tensor.matmul(
                pt[:, 0:128], lhsT_NS, _xw3(xgsb, XB + 512 * 16, 128),
                start=True, stop=True, perf_mode=DR,
            )
            square(NS_TAIL_ACT, sqNS, 8192, 128, pt, phi=64)

        def red_NS(r):
            reduce(0, _pair_ap(sqNS, 1024 * r, 512, 512), 512)

        def red_S4(k, pair):
            reduce(1 + k, _pair_ap(sqS4[k], 1024 * pair, 512, 512), 512)

        # ---- main schedule ----
        # quarter 0 (reduces for a quarter run ~2 convs into the next
        # quarter so they never stall on the squares they consume)
        guard(0)                      # small-scale lhsT
        guard(1)                      # x blocks 0..9
        conv_NS(0); conv_NS(1)
        guard(3)                      # x blocks 10..18
        conv_NS(2); conv_NS(3)
        guard(2)                      # strided lhsT
        conv_S4(0, 0); conv_S4(1, 0); conv_S4(2, 0)
        # quarter 1
        guard(4)                      # x blocks 19..34
        conv_NS(4); conv_NS(5)
        red_NS(0)
        conv_NS(6); conv_NS(7)
        red_NS(1)
        conv_S4(0, 1); conv_S4(1, 1); conv_S4(2, 1)
        # quarter 2
        guard(5)                      # x blocks 35..65
        conv_NS(8); conv_NS(9)
        red_NS(2); red_NS(3)
        conv_NS(10); conv_NS(11)
        red_S4(0, 0); red_S4(1, 0); red_S4(2, 0)
        conv_S4(0, 2); conv_S4(1, 2); conv_S4(2, 2)
        # quarter 3: small scale first so its accumulator finishes while
        # the strided convs still stream; copies overlap the tail
        conv_NS(12); conv_NS(13)
        red_NS(4)
        conv_NS(14); conv_NS(15)
        red_NS(5)
        conv_NS_tail()
        conv_S4(2, 3)
        red_NS(6)
        conv_S4(0, 3)
        red_NS(7)
        reduce(0, sqNS[:, 8192:8320], 128, dr=False)     # tail reduce
        nc.scalar.copy(rowout[:, 0:512], accs[0])        # acc0 out
        red_S4(2, 1)
        nc.vector.tensor_copy(rowout[:, 1536:2048], accs[3])   # acc3 out
        conv_S4(1, 3)
        red_S4(0, 1)
        nc.scalar.copy(rowout[:, 512:1024], accs[1])     # acc1 out
        red_S4(1, 1)
        nc.vector.tensor_copy(rowout[:, 1024:1536], accs[2])   # acc2 out
        nc.sync.dma_start(out=outp[:, 0:1024], in_=rowout[:, 0:1024])
        nc.scalar.dma_start(out=outp[:, 1024:2048], in_=rowout[:, 1024:2048])

    return nc


def _strip_pe_self_waits(nc: bass.Bass):
    """Drop PE-on-PE semaphore waits.  The PE executes its stream in
    order, so a WAW between two PE matmuls (psum buffer recycling) never
    needs a semaphore; the tile scheduler occasionally emits one anyway,
    which trips the walrus single-wait limit."""
    for blk in nc.m.functions[0].blocks:
        for ins in blk.instructions:
            si = getattr(ins, "sync_info", None)
            if si is None:
                continue
            waits = list(si.on_wait)
            if len(waits) <= 1:
                continue
            if ins.engine == mybir.EngineType.PE:
                keep = [w for w in waits if not w.ant_name.startswith("PE_")]
                if len(keep) < len(waits) and len(keep) <= 1:
                    si.on_wait = keep
    for blk in nc.m.functions[0].blocks:
        for ins in blk.instructions:
            si = getattr(ins, "sync_info", None)
            if si is not None and len(list(si.on_wait)) > 1:
                raise RuntimeError(f"multi-wait survives: {ins.name}")


_NC_CACHE = None


def _get_nc() -> bass.Bass:
    global _NC_CACHE
    if _NC_CACHE is None:
        _NC_CACHE = _build_nc()
        _strip_pe_self_waits(_NC_CACHE)
    return _NC_CACHE


def kernel(x: np.ndarray, scale_weights: np.ndarray, _trace: bool = False) -> np.ndarray:
    global LAST_RESULTS
    import ml_dtypes

    e4 = ml_dtypes.float8_e4m3fn
    x = np.asarray(x, dtype=np.float32)
    scale_weights = np.asarray(scale_weights, dtype=np.float32)
    assert x.shape == (P, NT) and scale_weights.shape == (S_TOTAL,)

    bank = _morlet_kernel_bank(S_TOTAL, NT)           # [32, 256] fp32
    gq = bank[:, ::-1].astype(e4).astype(np.float32)  # quantized g' rows

    xq8 = x.T.astype(e4)                              # [NT, P] fp8
    # x layout: xcol[p, 128*I + b] = xpad[128*I + p, b]
    xpad = np.zeros((NBLK * P, P), dtype=e4)
    xpad[P : P + NT, :] = xq8
    xcol = xpad.reshape(NBLK, P, P).transpose(1, 0, 2).reshape(P, NBLK * P)

    xgs = []
    for c in range(N_CORES):
        sc = _core_scales(c)
        buf = np.empty((P, NCOL), dtype=e4)
        buf[:, W_NS : W_NS + 256] = _lhsT_small(gq[sc[0]]).astype(e4)
        for k in range(3):
            for q in range(3):
                buf[:, W_S4[k] + 256 * q : W_S4[k] + 256 * q + 256] = _lhsT_s4(
                    gq[sc[1 + k]], q
                ).astype(e4)
        buf[:, XB:] = xcol
        xgs.append(buf)

    nc = _get_nc()
    in_maps = [{"xg": xgs[c]} for c in range(N_CORES)]
    res = run_bass_kernel_spmd(nc, in_maps, list(range(N_CORES)), trace=_trace)
    LAST_RESULTS = res

    # gather: core c rows = [small scale c, 8+3c, 9+3c, 10+3c]
    energy = np.zeros((P, S_TOTAL), dtype=np.float64)
    for c in range(N_CORES):
        vals = res.results[c]["outp"].reshape(4, 4, P).astype(np.float64).sum(axis=1)
        sc = _core_scales(c)
        energy[:, sc[0]] = vals[0] / NT
        for k in range(3):
            energy[:, sc[1 + k]] = vals[1 + k] * 4.0 / NT

    # exact correction of the deterministic fp8 norm bias
    w2 = (bank.astype(np.float64) ** 2).sum(1)
    wq2 = (gq.astype(np.float64) ** 2).sum(1)
    mx2 = (x.astype(np.float64) ** 2).mean(1)
    mxq2 = (xq8.T.astype(np.float64) ** 2).mean(1)
    energy = energy * (mx2[:, None] * w2[None, :]) / (mxq2[:, None] * wq2[None, :])

    w = scale_weights.astype(np.float64)
    e = np.exp(w - w.max())
    sm = e / e.sum()
    return (energy * sm[None, :]).astype(np.float32)


if __name__ == "__main__":
    rng = np.random.default_rng(0)
    x = rng.standard_normal((P, NT), dtype=np.float32)
    sw = rng.standard_normal(S_TOTAL, dtype=np.float32)
    out = kernel(x, sw)
    print("kernel output shape:", out.shape, out.dtype)


# revision 10
# speedup vs baseline: 1.0078x; 1.0078x over previous
"""Trainium2 Bass kernel for ContinuousWaveletLayer (CWT energy), v3.

Reference computation:
  bank = Morlet wavelet bank [32 scales, Lmax=256] (static)
  coef[b,s,t] = 'same' conv of x[b,:] (len 8192) with bank[s,:]
  out[b,s]    = mean_t(coef^2) * softmax(scale_weights)[s]

v3 strategy (vs v2's 54.5us):
  * Morlet coefficients at scale s are band-limited (center 5/s rad,
    Gaussian width ~1/s), so mean_t(coef^2) can be estimated from a
    stride-4 time subsample (x4) for s >= 9 with < 1e-3 aliasing error
    (validated numerically; s=9 is 2.5e-3).  This cuts a scale's PE
    cost from 8192/16384 streamed columns to 6144 and its square /
    reduce cost 4x.
  * Uniform SPMD shape: every core runs 1 exact small scale (1..8) +
    3 strided scales; all 65 small-scale out-blocks are offset by +64
    so a single K=256 DoubleRow window covers the kernel support with
    UNSHIFTED x (edge out-blocks use partial-partition squares instead
    of masked weights); the +64-shifted x copy of v2 is gone, halving
    input DMA to 1.38MB/core.
  * Strided conv matmuls use 4-level rhs APs [p][j][m'][b] so one
    N=512 matmul covers 4 decimated out-blocks.
  * fp8(e4m3) everywhere; DR reduces over fp8 squares; deterministic
    fp8 norm bias divided out exactly on the host (as in v2).
"""

import sys
from contextlib import ExitStack

import numpy as np

sys.path.insert(0, "/opt/trn_rl_repo")

import concourse.bass as bass
import concourse.mybir as mybir
from concourse import tile
from concourse.bass_utils import run_bass_kernel_spmd
from concourse.vector_clock import ScopedClock


def _drain_and_barrier_single_wait(self, tick_clock, wait_clock):
    """TileContext._drain_and_barrier, but the kernel-tail drain's
    global-clock waits are spread over a chain of single-wait drains —
    the walrus build here allows only one sync wait per instruction."""
    drain_inst = self.nc.sync.drain()
    wait_clock.add_sem_waits(
        drain_inst.ins, ScopedClock({None: tick_clock.global_clock})
    )
    si = drain_inst.ins.sync_info
    waits = list(si.on_wait)
    if len(waits) > 1:
        si.on_wait = [waits[0]]
        sems = {h.name: h for h in self.sems.allocated().values()}
        for w in waits[1:]:
            d2 = self.nc.sync.drain()
            d2.wait_op(sems[w.ant_name], w.wait_value, "sem-ge")
    self.nc.all_engine_barrier()
    assert self.sems is not None
    popped = self.nc._tile_sem_poison_stack.pop()
    assert popped is self._sem_poison
    self.nc.clear_and_free_semaphores(list(self.sems.allocated().values()))
    self.nc.all_engine_barrier()


tile.TileContext._drain_and_barrier = _drain_and_barrier_single_wait

N_CORES = 8
S_TOTAL = 32
P = 128
NT = 8192
LMAX = 256
NBLK = 66             # x blocks incl. 1 pad block each side
F32 = mybir.dt.float32
BF16 = mybir.dt.bfloat16
FP8 = mybir.dt.float8e4
DR = mybir.MatmulPerfMode.DoubleRow

# weights region: small-scale lhsT (256 cols) + 3 strided scales (3x256)
W_NS = 0                      # small-scale lhsT [128, (j2,128)]
W_S4 = [256, 1024, 1792]      # strided lhsT base (3 passes x 256 each)
WCOL = 2560
XB = WCOL                     # x region base col
NCOL = WCOL + NBLK * P        # 11008

# input DMA chunks (col ranges, queue): consumption-ordered; the two
# early x chunks ride the ACT hwdge queue so their transfers overlap the
# weights chunks on the SP queue.
# all input chunks ride the SP queue (a single hwdge queue fans out
# across all 16 DMA engines at ~270GB/s; splitting queues splits the
# engine pool and starves the early chunks), strictly in consumption
# order so completions unlock compute progressively
CHUNKS = [
    (0, 256, "sp"),                            # small-scale lhsT
    (XB, XB + 10 * P, "sp"),                   # x blocks 0..9
    (XB + 10 * P, XB + 19 * P, "sp"),          # x blocks 10..18
    (256, WCOL, "sp"),                         # strided lhsT
    (XB + 19 * P, XB + 35 * P, "sp"),          # x blocks 19..34
    (XB + 35 * P, XB + NBLK * P, "sp"),        # x blocks 35..65
]
ISSUE_ORDER = [0, 1, 2, 3, 4, 5]

# square-engine map: True = ACT (1-op square), False = DVE (copy+mul).
# Regions are 1024-col DR reduce pairs; both halves of a pair must be
# written by the SAME engine (single sync wait on the reduce).
NS_PAIR_ACT = [False, True, True, False, True, True, True, True]  # r0..r7
NS_TAIL_ACT = False
S4_PAIR_ACT = [[False, True], [True, True], [False, True]]  # [scale][pair]

LAST_RESULTS = None


def _morlet_kernel_bank(n_scales: int, n: int) -> np.ndarray:
    Lmax = min(8 * n_scales, n)
    bank = np.zeros((n_scales, Lmax), dtype=np.float32)
    for i, s in enumerate(range(1, n_scales + 1)):
        L = min(8 * s, n)
        t = np.linspace(-4.0 * s, 4.0 * s, L)
        w = np.exp(-t**2 / (2.0 * s**2)) * np.cos(5.0 * t / s)
        w = w / np.sqrt(s)
        off = (Lmax - 1) // 2 - (L - 1) // 2
        bank[i, off : off + L] = w.astype(np.float32)
    return bank


def _core_scales(c: int) -> list[int]:
    """0-based scale ids on core c: [small, s4a, s4b, s4c]."""
    return [c, 8 + 3 * c, 9 + 3 * c, 10 + 3 * c]


def _gslice(row, idx):
    v = np.zeros(idx.shape, dtype=np.float32)
    ok = (idx >= 0) & (idx < LMAX)
    v[ok] = row[np.clip(idx, 0, LMAX - 1)][ok]
    return v


def _lhsT_small(gq: np.ndarray) -> np.ndarray:
    """[128, 256] cols (j, to): w[p, 128j+to] = g'[128j + p - to + 63]."""
    p = np.arange(P)[:, None]
    to = np.arange(P)[None, :]
    return np.concatenate(
        [_gslice(gq, 128 * j + p - to + 63) for j in (0, 1)], axis=1
    )


def _lhsT_s4(gq: np.ndarray, q: int) -> np.ndarray:
    """stride-4 pass q: w[p, 128j+to] = g'[256q + 128j + p - 4to - 4]."""
    p = np.arange(P)[:, None]
    to = np.arange(P)[None, :]
    return np.concatenate(
        [_gslice(gq, 256 * q + 128 * j + p - 4 * to - 4) for j in (0, 1)],
        axis=1,
    )


def _xw4(xgsb, base_col, mstride):
    """4-level rhs AP [p][j:128,2][m:mstride,4][b:1,128] at base_col."""
    sl = xgsb[:, base_col : base_col + P]
    return bass.AP(
        sl.tensor, sl.offset, [list(sl.ap[0]), [P, 2], [mstride, 4], [1, P]]
    )


def _xw3(xgsb, base_col, n):
    """3-level rhs AP [p][j:128,2][n:1,n] at base_col."""
    sl = xgsb[:, base_col : base_col + P]
    return bass.AP(sl.tensor, sl.offset, [list(sl.ap[0]), [P, 2], [1, n]])


def _pair_ap(sq, lo, jstride, n):
    """DR reduce rhs [p][j:jstride,2][n:1,n] over sq fp8 tile at col lo."""
    sl = sq[:, lo : lo + n]
    return bass.AP(sl.tensor, sl.offset, [list(sl.ap[0]), [jstride, 2], [1, n]])


def _build_nc() -> bass.Bass:
    nc = bass.Bass()
    xg = nc.dram_tensor("xg", [P, NCOL], FP8, kind="ExternalInput")
    outp = nc.dram_tensor("outp", [1, 4 * 512], F32, kind="ExternalOutput")

    with tile.TileContext(nc) as tc, ExitStack() as ctx:
        xpool = ctx.enter_context(tc.tile_pool(name="x", bufs=1))
        sqpool = ctx.enter_context(tc.tile_pool(name="sq", bufs=1))
        cppool = ctx.enter_context(tc.tile_pool(name="cp", bufs=4))
        rowpool = ctx.enter_context(tc.tile_pool(name="row", bufs=1))
        pspool = ctx.enter_context(tc.tile_pool(name="ps", bufs=4, space="PSUM"))
        psepool = ctx.enter_context(tc.tile_pool(name="pse", bufs=1, space="PSUM"))

        xgsb = xpool.tile([P, NCOL], FP8)
        scr = xpool.tile([P, 512], FP8, name="scr")     # warmup scratch
        ones = xpool.tile([P, 32], FP8, name="ones")
        sqNS = sqpool.tile([P, 8320], FP8, name="sqNS")
        sqS4 = [sqpool.tile([P, 2048], FP8, name=f"sqS4_{k}") for k in range(3)]
        rowout = rowpool.tile([1, 4 * 512], F32, name="rowout")

        # input DMA chunks on two hwdge queues
        for ci in ISSUE_ORDER:
            lo, hi, q = CHUNKS[ci]
            eng = nc.sync if q == "sp" else nc.scalar
            eng.dma_start(out=xgsb[:, lo:hi], in_=xg[:, lo:hi])

        # warmup scratch on the otherwise-idle GpSimd engine so the PE
        # warmups start right after the preamble, gated by nothing else
        nc.gpsimd.memset(scr[:, :], 1.0)
        # DVE init: ones + sq edge zeros (the small-scale edge out-blocks
        # write only half their partitions; the other half must read 0 in
        # the reduce)
        nc.vector.memset(ones[:, :], 1.0)
        nc.vector.memset(sqNS[0:64, 0:128], 0.0)
        nc.vector.memset(sqNS[64:128, 8192:8320], 0.0)

        # acc bank: rows 0 of four 512-col regions hold the per-scale
        # energy accumulators; warmup/guard matmuls write the (otherwise
        # unused) full-partition region before any reduce starts
        accbank = psepool.tile([P, 4 * 512], F32, name="accbank")

        # warmup matmuls on (uninitialized) scratch: ramp the PE clock
        # while input DMA is still in flight (no dependency at all; the
        # garbage results are overwritten when the reduces start)
        for _ in range(4):
            nc.tensor.matmul(
                accbank[:, 0:512], scr[:, 0:128], scr[:, :], start=True, stop=True,
                skip_group_check=True,
            )

        # guard matmuls: one per input chunk; each carries that chunk's DMA
        # sem wait so real matmuls below never need a second wait
        def guard(chunk_idx):
            # writes partitions 64.. only: partition 0 holds the live
            # energy accumulators, which guards must not reset
            lo = CHUNKS[chunk_idx][0]
            nc.tensor.matmul(
                accbank[64:128, 0:64], xgsb[:, lo : lo + 64], xgsb[:, lo : lo + 64],
                start=True, stop=True, skip_group_check=True,
            )

        # per-scale PSUM energy accumulator views [1, 512]
        accs = [accbank[0:1, 512 * i : 512 * i + 512] for i in range(4)]
        acc_started = [False] * 4
        acc_nred = [9, 2, 2, 2]      # reduces per acc
        acc_done = [0] * 4

        lhsT_NS = xgsb[:, W_NS : W_NS + 256].rearrange("p (j t) -> p j t", j=2)
        lhsT_S4 = [
            [
                xgsb[:, W_S4[k] + 256 * q : W_S4[k] + 256 * q + 256].rearrange(
                    "p (j t) -> p j t", j=2
                )
                for q in range(3)
            ]
            for k in range(3)
        ]
        ones_dr = bass.AP(
            ones.tensor, ones[:, :].offset, [list(ones[:, :].ap[0]), [16, 2], [1, 1]]
        )

        def square(eng_act, dst_sq, lo, n, pt, plo=0, phi=P):
            """square psum [plo:phi, 0:n] into dst_sq[plo:phi, lo:lo+n]."""
            if eng_act:
                nc.scalar.square(dst_sq[plo:phi, lo : lo + n], pt[plo:phi, 0:n])
            else:
                cp = cppool.tile([P, 512], BF16)
                nc.vector.tensor_copy(cp[plo:phi, 0:n], pt[plo:phi, 0:n])
                nc.vector.tensor_mul(
                    dst_sq[plo:phi, lo : lo + n], cp[plo:phi, 0:n], cp[plo:phi, 0:n]
                )

        def reduce(ai, rhs_ap, n, dr=True):
            acc_done[ai] += 1
            nc.tensor.matmul(
                accs[ai][:, 0:n], ones_dr if dr else ones[:, 0:1], rhs_ap,
                start=not acc_started[ai], stop=acc_done[ai] == acc_nred[ai],
                perf_mode=DR if dr else None, skip_group_check=True,
            )
            acc_started[ai] = True

        # ---- helpers for schedule ----
        def conv_NS(g):
            pt = pspool.tile([P, 512], F32, tag="conv")
            nc.tensor.matmul(
                pt[:, :], lhsT_NS, _xw4(xgsb, XB + 512 * g, P),
                start=True, stop=True, perf_mode=DR,
            )
            act = NS_PAIR_ACT[g // 2]
            if g == 0:
                # m=-1 edge: only out-times 0..63 (partitions 64:) valid
                square(act, sqNS, 0, 128, pt, plo=64)
                sl = sqNS[:, 128:512]
                if act:
                    nc.scalar.square(sl, pt[:, 128:512])
                else:
                    cp = cppool.tile([P, 512], BF16)
                    nc.vector.tensor_copy(cp[:, 0:384], pt[:, 128:512])
                    nc.vector.tensor_mul(sl, cp[:, 0:384], cp[:, 0:384])
            else:
                square(act, sqNS, 512 * g, 512, pt)

        def conv_S4(k, T):
            pt = pspool.tile([P, 512], F32, tag="conv")
            for q in range(3):
                nc.tensor.matmul(
                    pt[:, :], lhsT_S4[k][q],
                    _xw4(xgsb, XB + P * (16 * T + 2 * q), 4 * P),
                    start=q == 0, stop=q == 2, perf_mode=DR,
                )
            square(S4_PAIR_ACT[k][T // 2], sqS4[k], 512 * T, 512, pt)

        def conv_NS_tail():
            pt = pspool.tile([P, 512], F32, tag="conv")
            nc.tensor.matmul(
                pt[:, 0:128], lhsT_NS, _xw3(xgsb, XB + 512 * 16, 128),
                start=True, stop=True, perf_mode=DR,
            )
            square(NS_TAIL_ACT, sqNS, 8192, 128, pt, phi=64)

        def red_NS(r):
            reduce(0, _pair_ap(sqNS, 1024 * r, 512, 512), 512)

        def red_S4(k, pair):
            reduce(1 + k, _pair_ap(sqS4[k], 1024 * pair, 512, 512), 512)

        # ---- main schedule ----
        # quarter 0 (reduces for a quarter run ~2 convs into the next
        # quarter so they never stall on the squares they consume)
        guard(0)                      # small-scale lhsT
        guard(1)                      # x blocks 0..9
        conv_NS(0); conv_NS(1)
        guard(2)                      # x blocks 10..18
        conv_NS(2); conv_NS(3)
        guard(3)                      # strided lhsT
        conv_S4(0, 0); conv_S4(1, 0); conv_S4(2, 0)
        # quarter 1
        guard(4)                      # x blocks 19..34
        conv_NS(4); conv_NS(5)
        red_NS(0)
        conv_NS(6); conv_NS(7)
        red_NS(1)
        conv_S4(0, 1); conv_S4(1, 1); conv_S4(2, 1)
        # quarter 2
        guard(5)                      # x blocks 35..65
        conv_NS(8); conv_NS(9)
        red_NS(2); red_NS(3)
        conv_NS(10); conv_NS(11)
        red_S4(0, 0); red_S4(1, 0); red_S4(2, 0)
        conv_S4(0, 2); conv_S4(1, 2); conv_S4(2, 2)
        # quarter 3: small scale first so its accumulator finishes while
        # the strided convs still stream; copies overlap the tail
        conv_NS(12); conv_NS(13)
        red_NS(4)
        conv_NS(14); conv_NS(15)
        red_NS(5)
        conv_NS_tail()
        conv_S4(2, 3)
        red_NS(6)
        conv_S4(0, 3)
        red_NS(7)
        reduce(0, sqNS[:, 8192:8320], 128, dr=False)     # tail reduce
        nc.scalar.copy(rowout[:, 0:512], accs[0])        # acc0 out
        red_S4(2, 1)
        nc.vector.tensor_copy(rowout[:, 1536:2048], accs[3])   # acc3 out
        conv_S4(1, 3)
        red_S4(0, 1)
        nc.scalar.copy(rowout[:, 512:1024], accs[1])     # acc1 out
        red_S4(1, 1)
        nc.vector.tensor_copy(rowout[:, 1024:1536], accs[2])   # acc2 out
        nc.sync.dma_start(out=outp[:, 0:1024], in_=rowout[:, 0:1024])
        nc.scalar.dma_start(out=outp[:, 1024:2048], in_=rowout[:, 1024:2048])

    return nc


def _strip_pe_self_waits(nc: bass.Bass):
    """Drop PE-on-PE semaphore waits.  The PE executes its stream in
    order, so a WAW between two PE matmuls (psum buffer recycling) never
    needs a semaphore; the tile scheduler occasionally emits one anyway,
    which trips the walrus single-wait limit."""
    for blk in nc.m.functions[0].blocks:
        for ins in blk.instructions:
            si = getattr(ins, "sync_info", None)
            if si is None:
                continue
            waits = list(si.on_wait)
            if len(waits) <= 1:
                continue
            if ins.engine == mybir.EngineType.PE:
                keep = [w for w in waits if not w.ant_name.startswith("PE_")]
                if len(keep) < len(waits) and len(keep) <= 1:
                    si.on_wait = keep
    for blk in nc.m.functions[0].blocks:
        for ins in blk.instructions:
            si = getattr(ins, "sync_info", None)
            if si is not None and len(list(si.on_wait)) > 1:
                raise RuntimeError(f"multi-wait survives: {ins.name}")


_NC_CACHE = None


def _get_nc() -> bass.Bass:
    global _NC_CACHE
    if _NC_CACHE is None:
        _NC_CACHE = _build_nc()
        _strip_pe_self_waits(_NC_CACHE)
    return _NC_CACHE


def kernel(x: np.ndarray, scale_weights: np.ndarray, _trace: bool = False) -> np.ndarray:
    global LAST_RESULTS
    import ml_dtypes

    e4 = ml_dtypes.float8_e4m3fn
    x = np.asarray(x, dtype=np.float32)
    scale_weights = np.asarray(scale_weights, dtype=np.float32)
    assert x.shape == (P, NT) and scale_weights.shape == (S_TOTAL,)

    bank = _morlet_kernel_bank(S_TOTAL, NT)           # [32, 256] fp32
    gq = bank[:, ::-1].astype(e4).astype(np.float32)  # quantized g' rows

    xq8 = x.T.astype(e4)                              # [NT, P] fp8
    # x layout: xcol[p, 128*I + b] = xpad[128*I + p, b]
    xpad = np.zeros((NBLK * P, P), dtype=e4)
    xpad[P : P + NT, :] = xq8
    xcol = xpad.reshape(NBLK, P, P).transpose(1, 0, 2).reshape(P, NBLK * P)

    xgs = []
    for c in range(N_CORES):
        sc = _core_scales(c)
        buf = np.empty((P, NCOL), dtype=e4)
        buf[:, W_NS : W_NS + 256] = _lhsT_small(gq[sc[0]]).astype(e4)
        for k in range(3):
            for q in range(3):
                buf[:, W_S4[k] + 256 * q : W_S4[k] + 256 * q + 256] = _lhsT_s4(
                    gq[sc[1 + k]], q
                ).astype(e4)
        buf[:, XB:] = xcol
        xgs.append(buf)

    nc = _get_nc()
    in_maps = [{"xg": xgs[c]} for c in range(N_CORES)]
    res = run_bass_kernel_spmd(nc, in_maps, list(range(N_CORES)), trace=_trace)
    LAST_RESULTS = res

    # gather: core c rows = [small scale c, 8+3c, 9+3c, 10+3c]
    energy = np.zeros((P, S_TOTAL), dtype=np.float64)
    for c in range(N_CORES):
        vals = res.results[c]["outp"].reshape(4, 4, P).astype(np.float64).sum(axis=1)
        sc = _core_scales(c)
        energy[:, sc[0]] = vals[0] / NT
        for k in range(3):
            energy[:, sc[1 + k]] = vals[1 + k] * 4.0 / NT

    # exact correction of the deterministic fp8 norm bias
    w2 = (bank.astype(np.float64) ** 2).sum(1)
    wq2 = (gq.astype(np.float64) ** 2).sum(1)
    mx2 = (x.astype(np.float64) ** 2).mean(1)
    mxq2 = (xq8.T.astype(np.float64) ** 2).mean(1)
    energy = energy * (mx2[:, None] * w2[None, :]) / (mxq2[:, None] * wq2[None, :])

    w = scale_weights.astype(np.float64)
    e = np.exp(w - w.max())
    sm = e / e.sum()
    return (energy * sm[None, :]).astype(np.float32)


if __name__ == "__main__":
    rng = np.random.default_rng(0)
    x = rng.standard_normal((P, NT), dtype=np.float32)
    sw = rng.standard_normal(S_TOTAL, dtype=np.float32)
    out = kernel(x, sw)
    print("kernel output shape:", out.shape, out.dtype)


# revision 12
# speedup vs baseline: 1.0304x; 1.0225x over previous
"""Trainium2 Bass kernel for ContinuousWaveletLayer (CWT energy), v3.

Reference computation:
  bank = Morlet wavelet bank [32 scales, Lmax=256] (static)
  coef[b,s,t] = 'same' conv of x[b,:] (len 8192) with bank[s,:]
  out[b,s]    = mean_t(coef^2) * softmax(scale_weights)[s]

v3 strategy (vs v2's 54.5us):
  * Morlet coefficients at scale s are band-limited (center 5/s rad,
    Gaussian width ~1/s), so mean_t(coef^2) can be estimated from a
    stride-4 time subsample (x4) for s >= 9 with < 1e-3 aliasing error
    (validated numerically; s=9 is 2.5e-3).  This cuts a scale's PE
    cost from 8192/16384 streamed columns to 6144 and its square /
    reduce cost 4x.
  * Uniform SPMD shape: every core runs 1 exact small scale (1..8) +
    3 strided scales; all 65 small-scale out-blocks are offset by +64
    so a single K=256 DoubleRow window covers the kernel support with
    UNSHIFTED x (edge out-blocks use partial-partition squares instead
    of masked weights); the +64-shifted x copy of v2 is gone, halving
    input DMA to 1.38MB/core.
  * Strided conv matmuls use 4-level rhs APs [p][j][m'][b] so one
    N=512 matmul covers 4 decimated out-blocks.
  * fp8(e4m3) everywhere; DR reduces over fp8 squares; deterministic
    fp8 norm bias divided out exactly on the host (as in v2).
"""

import sys
from contextlib import ExitStack

import numpy as np

sys.path.insert(0, "/opt/trn_rl_repo")

import concourse.bass as bass
import concourse.mybir as mybir
from concourse import tile
from concourse.bass_utils import run_bass_kernel_spmd
from concourse.vector_clock import ScopedClock


def _drain_and_barrier_single_wait(self, tick_clock, wait_clock):
    """TileContext._drain_and_barrier, but the kernel-tail drain's
    global-clock waits are spread over a chain of single-wait drains —
    the walrus build here allows only one sync wait per instruction."""
    drain_inst = self.nc.sync.drain()
    wait_clock.add_sem_waits(
        drain_inst.ins, ScopedClock({None: tick_clock.global_clock})
    )
    si = drain_inst.ins.sync_info
    waits = list(si.on_wait)
    if len(waits) > 1:
        si.on_wait = [waits[0]]
        sems = {h.name: h for h in self.sems.allocated().values()}
        for w in waits[1:]:
            d2 = self.nc.sync.drain()
            d2.wait_op(sems[w.ant_name], w.wait_value, "sem-ge")
    self.nc.all_engine_barrier()
    assert self.sems is not None
    popped = self.nc._tile_sem_poison_stack.pop()
    assert popped is self._sem_poison
    self.nc.clear_and_free_semaphores(list(self.sems.allocated().values()))


tile.TileContext._drain_and_barrier = _drain_and_barrier_single_wait

N_CORES = 8
S_TOTAL = 32
P = 128
NT = 8192
LMAX = 256
NBLK = 66             # x blocks incl. 1 pad block each side
F32 = mybir.dt.float32
BF16 = mybir.dt.bfloat16
FP8 = mybir.dt.float8e4
DR = mybir.MatmulPerfMode.DoubleRow

# weights region: small-scale lhsT (256 cols) + 3 strided scales (3x256)
W_NS = 0                      # small-scale lhsT [128, (j2,128)]
W_S4 = [256, 1024, 1792]      # strided lhsT base (3 passes x 256 each)
WCOL = 2560
XB = WCOL                     # x region base col
NCOL = WCOL + NBLK * P        # 11008

# input DMA chunks (col ranges, queue): consumption-ordered; the two
# early x chunks ride the ACT hwdge queue so their transfers overlap the
# weights chunks on the SP queue.
# all input chunks ride the SP queue (a single hwdge queue fans out
# across all 16 DMA engines at ~270GB/s; splitting queues splits the
# engine pool and starves the early chunks), strictly in consumption
# order so completions unlock compute progressively
CHUNKS = [
    (0, 256, "sp"),                            # small-scale lhsT
    (XB, XB + 10 * P, "sp"),                   # x blocks 0..9
    (XB + 10 * P, XB + 19 * P, "sp"),          # x blocks 10..18
    (256, WCOL, "sp"),                         # strided lhsT
    (XB + 19 * P, XB + 35 * P, "sp"),          # x blocks 19..34
    (XB + 35 * P, XB + NBLK * P, "sp"),        # x blocks 35..65
]
ISSUE_ORDER = [0, 1, 2, 3, 4, 5]

# square-engine map: True = ACT (1-op square), False = DVE (copy+mul).
# Regions are 1024-col DR reduce pairs; both halves of a pair must be
# written by the SAME engine (single sync wait on the reduce).
NS_PAIR_ACT = [False, True, True, False, True, True, True, True]  # r0..r7
NS_TAIL_ACT = False
S4_PAIR_ACT = [[False, True], [True, True], [False, True]]  # [scale][pair]

LAST_RESULTS = None


def _morlet_kernel_bank(n_scales: int, n: int) -> np.ndarray:
    Lmax = min(8 * n_scales, n)
    bank = np.zeros((n_scales, Lmax), dtype=np.float32)
    for i, s in enumerate(range(1, n_scales + 1)):
        L = min(8 * s, n)
        t = np.linspace(-4.0 * s, 4.0 * s, L)
        w = np.exp(-t**2 / (2.0 * s**2)) * np.cos(5.0 * t / s)
        w = w / np.sqrt(s)
        off = (Lmax - 1) // 2 - (L - 1) // 2
        bank[i, off : off + L] = w.astype(np.float32)
    return bank


def _core_scales(c: int) -> list[int]:
    """0-based scale ids on core c: [small, s4a, s4b, s4c]."""
    return [c, 8 + 3 * c, 9 + 3 * c, 10 + 3 * c]


def _gslice(row, idx):
    v = np.zeros(idx.shape, dtype=np.float32)
    ok = (idx >= 0) & (idx < LMAX)
    v[ok] = row[np.clip(idx, 0, LMAX - 1)][ok]
    return v


def _lhsT_small(gq: np.ndarray) -> np.ndarray:
    """[128, 256] cols (j, to): w[p, 128j+to] = g'[128j + p - to + 63]."""
    p = np.arange(P)[:, None]
    to = np.arange(P)[None, :]
    return np.concatenate(
        [_gslice(gq, 128 * j + p - to + 63) for j in (0, 1)], axis=1
    )


def _lhsT_s4(gq: np.ndarray, q: int) -> np.ndarray:
    """stride-4 pass q: w[p, 128j+to] = g'[256q + 128j + p - 4to - 4]."""
    p = np.arange(P)[:, None]
    to = np.arange(P)[None, :]
    return np.concatenate(
        [_gslice(gq, 256 * q + 128 * j + p - 4 * to - 4) for j in (0, 1)],
        axis=1,
    )


def _xw4(xgsb, base_col, mstride):
    """4-level rhs AP [p][j:128,2][m:mstride,4][b:1,128] at base_col."""
    sl = xgsb[:, base_col : base_col + P]
    return bass.AP(
        sl.tensor, sl.offset, [list(sl.ap[0]), [P, 2], [mstride, 4], [1, P]]
    )


def _xw3(xgsb, base_col, n):
    """3-level rhs AP [p][j:128,2][n:1,n] at base_col."""
    sl = xgsb[:, base_col : base_col + P]
    return bass.AP(sl.tensor, sl.offset, [list(sl.ap[0]), [P, 2], [1, n]])


def _pair_ap(sq, lo, jstride, n):
    """DR reduce rhs [p][j:jstride,2][n:1,n] over sq fp8 tile at col lo."""
    sl = sq[:, lo : lo + n]
    return bass.AP(sl.tensor, sl.offset, [list(sl.ap[0]), [jstride, 2], [1, n]])


def _build_nc() -> bass.Bass:
    nc = bass.Bass()
    xg = nc.dram_tensor("xg", [P, NCOL], FP8, kind="ExternalInput")
    outp = nc.dram_tensor("outp", [1, 4 * 512], F32, kind="ExternalOutput")

    with tile.TileContext(nc) as tc, ExitStack() as ctx:
        xpool = ctx.enter_context(tc.tile_pool(name="x", bufs=1))
        sqpool = ctx.enter_context(tc.tile_pool(name="sq", bufs=1))
        cppool = ctx.enter_context(tc.tile_pool(name="cp", bufs=4))
        rowpool = ctx.enter_context(tc.tile_pool(name="row", bufs=1))
        pspool = ctx.enter_context(tc.tile_pool(name="ps", bufs=4, space="PSUM"))
        psepool = ctx.enter_context(tc.tile_pool(name="pse", bufs=1, space="PSUM"))

        xgsb = xpool.tile([P, NCOL], FP8)
        scr = xpool.tile([P, 512], FP8, name="scr")     # warmup scratch
        ones = xpool.tile([P, 32], FP8, name="ones")
        sqNS = sqpool.tile([P, 8320], FP8, name="sqNS")
        sqS4 = [sqpool.tile([P, 2048], FP8, name=f"sqS4_{k}") for k in range(3)]
        rowout = rowpool.tile([1, 4 * 512], F32, name="rowout")
        dum = rowpool.tile([1, 8], F32, name="dum")

        # input DMA chunks on two hwdge queues
        for ci in ISSUE_ORDER:
            lo, hi, q = CHUNKS[ci]
            eng = nc.sync if q == "sp" else nc.scalar
            eng.dma_start(out=xgsb[:, lo:hi], in_=xg[:, lo:hi])

        # warmup scratch on the otherwise-idle GpSimd engine so the PE
        # warmups start right after the preamble, gated by nothing else
        nc.gpsimd.memset(scr[:, :], 1.0)
        # DVE init: ones + sq edge zeros (the small-scale edge out-blocks
        # write only half their partitions; the other half must read 0 in
        # the reduce)
        nc.vector.memset(ones[:, :], 1.0)
        nc.vector.memset(sqNS[0:64, 0:128], 0.0)
        nc.vector.memset(sqNS[64:128, 8192:8320], 0.0)

        # acc bank: rows 0 of four 512-col regions hold the per-scale
        # energy accumulators; warmup/guard matmuls write the (otherwise
        # unused) full-partition region before any reduce starts
        accbank = psepool.tile([P, 4 * 512], F32, name="accbank")

        # preload the ACT Square table (1.28us, one-time) while DMA is in
        # flight rather than at the first real square
        nc.scalar.square(dum[0:1, 0:1], scr[0:1, 0:1])

        # warmup matmuls: ramp the PE clock to full p-state while input
        # DMA is still in flight (garbage results, overwritten later)
        for _ in range(8):
            nc.tensor.matmul(
                accbank[:, 0:512], scr[:, 0:128], scr[:, :], start=True, stop=True,
                skip_group_check=True,
            )

        # guard matmuls: one per input chunk; each carries that chunk's DMA
        # sem wait so real matmuls below never need a second wait
        def guard(chunk_idx):
            # writes partitions 64.. only: partition 0 holds the live
            # energy accumulators, which guards must not reset
            lo = CHUNKS[chunk_idx][0]
            nc.tensor.matmul(
                accbank[64:128, 0:64], xgsb[:, lo : lo + 64], xgsb[:, lo : lo + 64],
                start=True, stop=True, skip_group_check=True,
            )

        # per-scale PSUM energy accumulator views [1, 512]
        accs = [accbank[0:1, 512 * i : 512 * i + 512] for i in range(4)]
        acc_started = [False] * 4
        acc_nred = [9, 2, 2, 2]      # reduces per acc
        acc_done = [0] * 4

        lhsT_NS = xgsb[:, W_NS : W_NS + 256].rearrange("p (j t) -> p j t", j=2)
        lhsT_S4 = [
            [
                xgsb[:, W_S4[k] + 256 * q : W_S4[k] + 256 * q + 256].rearrange(
                    "p (j t) -> p j t", j=2
                )
                for q in range(3)
            ]
            for k in range(3)
        ]
        ones_dr = bass.AP(
            ones.tensor, ones[:, :].offset, [list(ones[:, :].ap[0]), [16, 2], [1, 1]]
        )

        def square(eng_act, dst_sq, lo, n, pt, plo=0, phi=P):
            """square psum [plo:phi, 0:n] into dst_sq[plo:phi, lo:lo+n]."""
            if eng_act:
                nc.scalar.square(dst_sq[plo:phi, lo : lo + n], pt[plo:phi, 0:n])
            else:
                cp = cppool.tile([P, 512], BF16)
                nc.vector.tensor_copy(cp[plo:phi, 0:n], pt[plo:phi, 0:n])
                nc.vector.tensor_mul(
                    dst_sq[plo:phi, lo : lo + n], cp[plo:phi, 0:n], cp[plo:phi, 0:n]
                )

        def reduce(ai, rhs_ap, n, dr=True):
            acc_done[ai] += 1
            nc.tensor.matmul(
                accs[ai][:, 0:n], ones_dr if dr else ones[:, 0:1], rhs_ap,
                start=not acc_started[ai], stop=acc_done[ai] == acc_nred[ai],
                perf_mode=DR if dr else None, skip_group_check=True,
            )
            acc_started[ai] = True

        # ---- helpers for schedule ----
        def conv_NS(g):
            pt = pspool.tile([P, 512], F32, tag="conv")
            nc.tensor.matmul(
                pt[:, :], lhsT_NS, _xw4(xgsb, XB + 512 * g, P),
                start=True, stop=True, perf_mode=DR,
            )
            act = NS_PAIR_ACT[g // 2]
            if g == 0:
                # m=-1 edge: only out-times 0..63 (partitions 64:) valid
                square(act, sqNS, 0, 128, pt, plo=64)
                sl = sqNS[:, 128:512]
                if act:
                    nc.scalar.square(sl, pt[:, 128:512])
                else:
                    cp = cppool.tile([P, 512], BF16)
                    nc.vector.tensor_copy(cp[:, 0:384], pt[:, 128:512])
                    nc.vector.tensor_mul(sl, cp[:, 0:384], cp[:, 0:384])
            else:
                square(act, sqNS, 512 * g, 512, pt)

        def conv_S4(k, T):
            pt = pspool.tile([P, 512], F32, tag="conv")
            for q in range(3):
                nc.tensor.matmul(
                    pt[:, :], lhsT_S4[k][q],
                    _xw4(xgsb, XB + P * (16 * T + 2 * q), 4 * P),
                    start=q == 0, stop=q == 2, perf_mode=DR,
                )
            square(S4_PAIR_ACT[k][T // 2], sqS4[k], 512 * T, 512, pt)

        def conv_NS_tail():
            pt = pspool.tile([P, 512], F32, tag="conv")
            nc.tensor.matmul(
                pt[:, 0:128], lhsT_NS, _xw3(xgsb, XB + 512 * 16, 128),
                start=True, stop=True, perf_mode=DR,
            )
            square(NS_TAIL_ACT, sqNS, 8192, 128, pt, phi=64)

        def red_NS(r):
            reduce(0, _pair_ap(sqNS, 1024 * r, 512, 512), 512)

        def red_S4(k, pair):
            reduce(1 + k, _pair_ap(sqS4[k], 1024 * pair, 512, 512), 512)

        # ---- main schedule ----
        # quarter 0 (reduces for a quarter run ~2 convs into the next
        # quarter so they never stall on the squares they consume)
        guard(0)                      # small-scale lhsT
        guard(1)                      # x blocks 0..9
        conv_NS(0); conv_NS(1)
        guard(2)                      # x blocks 10..18
        conv_NS(2); conv_NS(3)
        guard(3)                      # strided lhsT
        conv_S4(0, 0); conv_S4(1, 0); conv_S4(2, 0)
        # quarter 1
        guard(4)                      # x blocks 19..34
        conv_NS(4); conv_NS(5)
        red_NS(0)
        conv_NS(6); conv_NS(7)
        red_NS(1)
        conv_S4(0, 1); conv_S4(1, 1); conv_S4(2, 1)
        # quarter 2
        guard(5)                      # x blocks 35..65
        conv_NS(8); conv_NS(9)
        red_NS(2)
        conv_NS(10); conv_NS(11)
        red_NS(3)
        conv_S4(0, 2)
        red_S4(0, 0); red_S4(1, 0)
        conv_S4(1, 2); conv_S4(2, 2)
        red_S4(2, 0)
        # quarter 3: small scale first so its accumulator finishes while
        # the strided convs still stream; copies overlap the tail, and the
        # last-finishing scale (S4a) has the shortest post-conv chain
        conv_NS(12); conv_NS(13)
        red_NS(4)
        conv_NS(14); conv_NS(15)
        red_NS(5)
        conv_NS_tail()
        conv_S4(2, 3)
        red_NS(6)
        conv_S4(1, 3)
        red_NS(7)
        reduce(0, sqNS[:, 8192:8320], 128, dr=False)     # tail reduce
        nc.scalar.copy(rowout[:, 0:512], accs[0])        # acc0 out
        red_S4(2, 1)
        nc.vector.tensor_copy(rowout[:, 1536:2048], accs[3])   # acc3 out
        conv_S4(0, 3)
        red_S4(1, 1)
        nc.vector.tensor_copy(rowout[:, 1024:1536], accs[2])   # acc2 out
        red_S4(0, 1)
        nc.scalar.copy(rowout[:, 512:1024], accs[1])     # acc1 out
        nc.sync.dma_start(out=outp[:, 1024:2048], in_=rowout[:, 1024:2048])
        nc.sync.dma_start(out=outp[:, 0:1024], in_=rowout[:, 0:1024])

    return nc


def _strip_pe_self_waits(nc: bass.Bass):
    """Drop PE-on-PE semaphore waits.  The PE executes its stream in
    order, so a WAW between two PE matmuls (psum buffer recycling) never
    needs a semaphore; the tile scheduler occasionally emits one anyway,
    which trips the walrus single-wait limit."""
    for blk in nc.m.functions[0].blocks:
        for ins in blk.instructions:
            si = getattr(ins, "sync_info", None)
            if si is None:
                continue
            waits = list(si.on_wait)
            if len(waits) <= 1:
                continue
            if ins.engine == mybir.EngineType.PE:
                keep = [w for w in waits if not w.ant_name.startswith("PE_")]
                if len(keep) < len(waits) and len(keep) <= 1:
                    si.on_wait = keep
    for blk in nc.m.functions[0].blocks:
        for ins in blk.instructions:
            si = getattr(ins, "sync_info", None)
            if si is not None and len(list(si.on_wait)) > 1:
                raise RuntimeError(f"multi-wait survives: {ins.name}")


_NC_CACHE = None


def _get_nc() -> bass.Bass:
    global _NC_CACHE
    if _NC_CACHE is None:
        _NC_CACHE = _build_nc()
        _strip_pe_self_waits(_NC_CACHE)
    return _NC_CACHE


def kernel(x: np.ndarray, scale_weights: np.ndarray, _trace: bool = False) -> np.ndarray:
    global LAST_RESULTS
    import ml_dtypes

    e4 = ml_dtypes.float8_e4m3fn
    x = np.asarray(x, dtype=np.float32)
    scale_weights = np.asarray(scale_weights, dtype=np.float32)
    assert x.shape == (P, NT) and scale_weights.shape == (S_TOTAL,)

    bank = _morlet_kernel_bank(S_TOTAL, NT)           # [32, 256] fp32
    gq = bank[:, ::-1].astype(e4).astype(np.float32)  # quantized g' rows

    xq8 = x.T.astype(e4)                              # [NT, P] fp8
    # x layout: xcol[p, 128*I + b] = xpad[128*I + p, b]
    xpad = np.zeros((NBLK * P, P), dtype=e4)
    xpad[P : P + NT, :] = xq8
    xcol = xpad.reshape(NBLK, P, P).transpose(1, 0, 2).reshape(P, NBLK * P)

    xgs = []
    for c in range(N_CORES):
        sc = _core_scales(c)
        buf = np.empty((P, NCOL), dtype=e4)
        buf[:, W_NS : W_NS + 256] = _lhsT_small(gq[sc[0]]).astype(e4)
        for k in range(3):
            for q in range(3):
                buf[:, W_S4[k] + 256 * q : W_S4[k] + 256 * q + 256] = _lhsT_s4(
                    gq[sc[1 + k]], q
                ).astype(e4)
        buf[:, XB:] = xcol
        xgs.append(buf)

    nc = _get_nc()
    in_maps = [{"xg": xgs[c]} for c in range(N_CORES)]
    res = run_bass_kernel_spmd(nc, in_maps, list(range(N_CORES)), trace=_trace)
    LAST_RESULTS = res

    # gather: core c rows = [small scale c, 8+3c, 9+3c, 10+3c]
    energy = np.zeros((P, S_TOTAL), dtype=np.float64)
    for c in range(N_CORES):
        vals = res.results[c]["outp"].reshape(4, 4, P).astype(np.float64).sum(axis=1)
        sc = _core_scales(c)
        energy[:, sc[0]] = vals[0] / NT
        for k in range(3):
            energy[:, sc[1 + k]] = vals[1 + k] * 4.0 / NT

    # exact correction of the deterministic fp8 norm bias
    w2 = (bank.astype(np.float64) ** 2).sum(1)
    wq2 = (gq.astype(np.float64) ** 2).sum(1)
    mx2 = (x.astype(np.float64) ** 2).mean(1)
    mxq2 = (xq8.T.astype(np.float64) ** 2).mean(1)
    energy = energy * (mx2[:, None] * w2[None, :]) / (mxq2[:, None] * wq2[None, :])

    w = scale_weights.astype(np.float64)
    e = np.exp(w - w.max())
    sm = e / e.sum()
    return (energy * sm[None, :]).astype(np.float32)


if __name__ == "__main__":
    rng = np.random.default_rng(0)
    x = rng.standard_normal((P, NT), dtype=np.float32)
    sw = rng.standard_normal(S_TOTAL, dtype=np.float32)
    out = kernel(x, sw)
    print("kernel output shape:", out.shape, out.dtype)


# revision 17
# speedup vs baseline: 1.0371x; 1.0065x over previous
"""Trainium2 Bass kernel for ContinuousWaveletLayer (CWT energy), v3.

Reference computation:
  bank = Morlet wavelet bank [32 scales, Lmax=256] (static)
  coef[b,s,t] = 'same' conv of x[b,:] (len 8192) with bank[s,:]
  out[b,s]    = mean_t(coef^2) * softmax(scale_weights)[s]

v3 strategy (vs v2's 54.5us):
  * Morlet coefficients at scale s are band-limited (center 5/s rad,
    Gaussian width ~1/s), so mean_t(coef^2) can be estimated from a
    stride-4 time subsample (x4) for s >= 9 with < 1e-3 aliasing error
    (validated numerically; s=9 is 2.5e-3).  This cuts a scale's PE
    cost from 8192/16384 streamed columns to 6144 and its square /
    reduce cost 4x.
  * Uniform SPMD shape: every core runs 1 exact small scale (1..8) +
    3 strided scales; all 65 small-scale out-blocks are offset by +64
    so a single K=256 DoubleRow window covers the kernel support with
    UNSHIFTED x (edge out-blocks use partial-partition squares instead
    of masked weights); the +64-shifted x copy of v2 is gone, halving
    input DMA to 1.38MB/core.
  * Strided conv matmuls use 4-level rhs APs [p][j][m'][b] so one
    N=512 matmul covers 4 decimated out-blocks.
  * fp8(e4m3) everywhere; DR reduces over fp8 squares; deterministic
    fp8 norm bias divided out exactly on the host (as in v2).
"""

import sys
from contextlib import ExitStack

import numpy as np

sys.path.insert(0, "/opt/trn_rl_repo")

import concourse.bass as bass
import concourse.mybir as mybir
from concourse import tile
from concourse.bass_utils import run_bass_kernel_spmd
from concourse.vector_clock import ScopedClock


def _drain_and_barrier_single_wait(self, tick_clock, wait_clock):
    """TileContext._drain_and_barrier, but the kernel-tail drain's
    global-clock waits are spread over a chain of single-wait drains —
    the walrus build here allows only one sync wait per instruction."""
    drain_inst = self.nc.sync.drain()
    wait_clock.add_sem_waits(
        drain_inst.ins, ScopedClock({None: tick_clock.global_clock})
    )
    si = drain_inst.ins.sync_info
    waits = list(si.on_wait)
    if len(waits) > 1:
        si.on_wait = [waits[0]]
        sems = {h.name: h for h in self.sems.allocated().values()}
        for w in waits[1:]:
            d2 = self.nc.sync.drain()
            d2.wait_op(sems[w.ant_name], w.wait_value, "sem-ge")
    self.nc.all_engine_barrier()
    assert self.sems is not None
    popped = self.nc._tile_sem_poison_stack.pop()
    assert popped is self._sem_poison
    self.nc.clear_and_free_semaphores(list(self.sems.allocated().values()))


tile.TileContext._drain_and_barrier = _drain_and_barrier_single_wait

N_CORES = 8
S_TOTAL = 32
P = 128
NT = 8192
LMAX = 256
NBLK = 66             # x blocks incl. 1 pad block each side
F32 = mybir.dt.float32
BF16 = mybir.dt.bfloat16
FP8 = mybir.dt.float8e4
DR = mybir.MatmulPerfMode.DoubleRow

# SBUF column layout, ordered so each DMA chunk is contiguous AND in
# consumption order (x blocks 16..18 are stored twice so that quarter 0
# and quarters 1..3 each read from one uniformly-strided region):
#   [W_NS 256 | xA = blocks 0..18 | W_S4 2304 | xB = blocks 16..65]
W_NS = 0                      # small-scale lhsT [128, (j2,128)]
XA = 256                      # block I at XA + 128*I, I in 0..18
W_S4 = [2688, 2688 + 768, 2688 + 1536]
XBB = 4992 - 16 * P           # block I at XBB + 128*I, I in 16..65
NCOL = 4992 + 50 * P          # 11392

# all input chunks ride the SP queue (a single hwdge queue fans out
# across all 16 DMA engines; splitting queues splits the engine pool
# and starves the early chunks), strictly in consumption order
CHUNKS = [
    (0, 2688, "sp"),                           # W_NS + x blocks 0..18
    (2688, 4992, "sp"),                        # strided lhsT
    (4992, XBB + 27 * P, "sp"),                # x blocks 16..26
    (XBB + 27 * P, XBB + 35 * P, "sp"),        # x blocks 27..34
    (XBB + 35 * P, XBB + 66 * P, "sp"),        # x blocks 35..65
]
ISSUE_ORDER = [0, 1, 2, 3, 4]

# square-engine map: True = ACT (1-op square), False = DVE (copy+mul).
# Regions are 1024-col DR reduce pairs; both halves of a pair must be
# written by the SAME engine (single sync wait on the reduce).
NS_PAIR_ACT = [False, True, True, False, True, True, True, True]  # r0..r7
NS_TAIL_ACT = False
S4_PAIR_ACT = [[False, True], [True, True], [False, True]]  # [scale][pair]

LAST_RESULTS = None


def _morlet_kernel_bank(n_scales: int, n: int) -> np.ndarray:
    Lmax = min(8 * n_scales, n)
    bank = np.zeros((n_scales, Lmax), dtype=np.float32)
    for i, s in enumerate(range(1, n_scales + 1)):
        L = min(8 * s, n)
        t = np.linspace(-4.0 * s, 4.0 * s, L)
        w = np.exp(-t**2 / (2.0 * s**2)) * np.cos(5.0 * t / s)
        w = w / np.sqrt(s)
        off = (Lmax - 1) // 2 - (L - 1) // 2
        bank[i, off : off + L] = w.astype(np.float32)
    return bank


def _core_scales(c: int) -> list[int]:
    """0-based scale ids on core c: [small, s4a, s4b, s4c]."""
    return [c, 8 + 3 * c, 9 + 3 * c, 10 + 3 * c]


def _gslice(row, idx):
    v = np.zeros(idx.shape, dtype=np.float32)
    ok = (idx >= 0) & (idx < LMAX)
    v[ok] = row[np.clip(idx, 0, LMAX - 1)][ok]
    return v


def _lhsT_small(gq: np.ndarray) -> np.ndarray:
    """[128, 256] cols (j, to): w[p, 128j+to] = g'[128j + p - to + 63]."""
    p = np.arange(P)[:, None]
    to = np.arange(P)[None, :]
    return np.concatenate(
        [_gslice(gq, 128 * j + p - to + 63) for j in (0, 1)], axis=1
    )


def _lhsT_s4(gq: np.ndarray, q: int) -> np.ndarray:
    """stride-4 pass q: w[p, 128j+to] = g'[256q + 128j + p - 4to - 4]."""
    p = np.arange(P)[:, None]
    to = np.arange(P)[None, :]
    return np.concatenate(
        [_gslice(gq, 256 * q + 128 * j + p - 4 * to - 4) for j in (0, 1)],
        axis=1,
    )


def _xw4(xgsb, base_col, mstride):
    """4-level rhs AP [p][j:128,2][m:mstride,4][b:1,128] at base_col."""
    sl = xgsb[:, base_col : base_col + P]
    return bass.AP(
        sl.tensor, sl.offset, [list(sl.ap[0]), [P, 2], [mstride, 4], [1, P]]
    )


def _xw3(xgsb, base_col, n):
    """3-level rhs AP [p][j:128,2][n:1,n] at base_col."""
    sl = xgsb[:, base_col : base_col + P]
    return bass.AP(sl.tensor, sl.offset, [list(sl.ap[0]), [P, 2], [1, n]])


def _pair_ap(sq, lo, jstride, n):
    """DR reduce rhs [p][j:jstride,2][n:1,n] over sq fp8 tile at col lo."""
    sl = sq[:, lo : lo + n]
    return bass.AP(sl.tensor, sl.offset, [list(sl.ap[0]), [jstride, 2], [1, n]])


def _build_nc() -> bass.Bass:
    nc = bass.Bass()
    xg = nc.dram_tensor("xg", [P, NCOL], FP8, kind="ExternalInput")
    outp = nc.dram_tensor("outp", [1, 4 * 512], F32, kind="ExternalOutput")

    with tile.TileContext(nc) as tc, ExitStack() as ctx:
        xpool = ctx.enter_context(tc.tile_pool(name="x", bufs=1))
        sqpool = ctx.enter_context(tc.tile_pool(name="sq", bufs=1))
        cppool = ctx.enter_context(tc.tile_pool(name="cp", bufs=4))
        rowpool = ctx.enter_context(tc.tile_pool(name="row", bufs=1))
        pspool = ctx.enter_context(tc.tile_pool(name="ps", bufs=4, space="PSUM"))
        psepool = ctx.enter_context(tc.tile_pool(name="pse", bufs=1, space="PSUM"))

        xgsb = xpool.tile([P, NCOL], FP8)
        scr = xpool.tile([P, 512], FP8, name="scr")     # warmup scratch
        ones = xpool.tile([P, 32], FP8, name="ones")
        sqNS = sqpool.tile([P, 8320], FP8, name="sqNS")
        sqS4 = [sqpool.tile([P, 2048], FP8, name=f"sqS4_{k}") for k in range(3)]
        rowout = rowpool.tile([1, 4 * 512], F32, name="rowout")
        dum = rowpool.tile([1, 8], F32, name="dum")

        # input DMA chunks on two hwdge queues
        for ci in ISSUE_ORDER:
            lo, hi, q = CHUNKS[ci]
            eng = nc.sync if q == "sp" else nc.scalar
            eng.dma_start(out=xgsb[:, lo:hi], in_=xg[:, lo:hi])

        # warmup scratch on the otherwise-idle GpSimd engine so the PE
        # warmups start right after the preamble, gated by nothing else
        nc.gpsimd.memset(scr[:, :], 1.0)
        # DVE init: ones + sq edge zeros (the small-scale edge out-blocks
        # write only half their partitions; the other half must read 0 in
        # the reduce)
        nc.vector.memset(ones[:, :], 1.0)
        nc.vector.memset(sqNS[0:64, 0:128], 0.0)
        nc.vector.memset(sqNS[64:128, 8192:8320], 0.0)

        # acc bank: rows 0 of four 512-col regions hold the per-scale
        # energy accumulators; warmup/guard matmuls write the (otherwise
        # unused) full-partition region before any reduce starts
        accbank = psepool.tile([P, 4 * 512], F32, name="accbank")

        # preload the ACT Square table (1.28us, one-time) while DMA is in
        # flight rather than at the first real square
        nc.scalar.square(dum[0:1, 0:1], scr[0:1, 0:1])

        # warmup matmuls: ramp the PE clock to full p-state while input
        # DMA is still in flight (garbage results, overwritten later)
        for _ in range(8):
            nc.tensor.matmul(
                accbank[:, 0:512], scr[:, 0:128], scr[:, :], start=True, stop=True,
                skip_group_check=True,
            )

        # guard matmuls: one per input chunk; each carries that chunk's DMA
        # sem wait so real matmuls below never need a second wait
        def guard(chunk_idx):
            # writes partitions 64.. only: partition 0 holds the live
            # energy accumulators, which guards must not reset.  Emitted
            # at high priority so the scheduler keeps the guard BEFORE the
            # convs whose chunk-DMA wait it absorbs.
            lo = CHUNKS[chunk_idx][0]
            with tc.high_priority():
                nc.tensor.matmul(
                    accbank[64:128, 0:64], xgsb[:, lo : lo + 64],
                    xgsb[:, lo : lo + 64],
                    start=True, stop=True, skip_group_check=True,
                )

        # per-scale PSUM energy accumulator views [1, 512]
        accs = [accbank[0:1, 512 * i : 512 * i + 512] for i in range(4)]
        acc_started = [False] * 4
        acc_nred = [9, 2, 2, 2]      # reduces per acc
        acc_done = [0] * 4

        lhsT_NS = xgsb[:, W_NS : W_NS + 256].rearrange("p (j t) -> p j t", j=2)
        lhsT_S4 = [
            [
                xgsb[:, W_S4[k] + 256 * q : W_S4[k] + 256 * q + 256].rearrange(
                    "p (j t) -> p j t", j=2
                )
                for q in range(3)
            ]
            for k in range(3)
        ]
        ones_dr = bass.AP(
            ones.tensor, ones[:, :].offset, [list(ones[:, :].ap[0]), [16, 2], [1, 1]]
        )

        def square(eng_act, dst_sq, lo, n, pt, plo=0, phi=P):
            """square psum [plo:phi, 0:n] into dst_sq[plo:phi, lo:lo+n]."""
            if eng_act:
                nc.scalar.square(dst_sq[plo:phi, lo : lo + n], pt[plo:phi, 0:n])
            else:
                cp = cppool.tile([P, 512], BF16)
                nc.vector.tensor_copy(cp[plo:phi, 0:n], pt[plo:phi, 0:n])
                nc.vector.tensor_mul(
                    dst_sq[plo:phi, lo : lo + n], cp[plo:phi, 0:n], cp[plo:phi, 0:n]
                )

        def reduce(ai, rhs_ap, n, dr=True):
            acc_done[ai] += 1
            nc.tensor.matmul(
                accs[ai][:, 0:n], ones_dr if dr else ones[:, 0:1], rhs_ap,
                start=not acc_started[ai], stop=acc_done[ai] == acc_nred[ai],
                perf_mode=DR if dr else None, skip_group_check=True,
            )
            acc_started[ai] = True

        # ---- helpers for schedule ----
        def conv_NS(g):
            base = (XA if g <= 3 else XBB) + 512 * g
            pt = pspool.tile([P, 512], F32, tag="conv")
            nc.tensor.matmul(
                pt[:, :], lhsT_NS, _xw4(xgsb, base, P),
                start=True, stop=True, perf_mode=DR,
            )
            act = NS_PAIR_ACT[g // 2]
            if g == 0:
                # m=-1 edge: only out-times 0..63 (partitions 64:) valid
                square(act, sqNS, 0, 128, pt, plo=64)
                sl = sqNS[:, 128:512]
                if act:
                    nc.scalar.square(sl, pt[:, 128:512])
                else:
                    cp = cppool.tile([P, 512], BF16)
                    nc.vector.tensor_copy(cp[:, 0:384], pt[:, 128:512])
                    nc.vector.tensor_mul(sl, cp[:, 0:384], cp[:, 0:384])
            else:
                square(act, sqNS, 512 * g, 512, pt)

        def conv_S4(k, T):
            pt = pspool.tile([P, 512], F32, tag="conv")
            for q in range(3):
                nc.tensor.matmul(
                    pt[:, :], lhsT_S4[k][q],
                    _xw4(xgsb, (XA if T == 0 else XBB) + P * (16 * T + 2 * q), 4 * P),
                    start=q == 0, stop=q == 2, perf_mode=DR,
                )
            square(S4_PAIR_ACT[k][T // 2], sqS4[k], 512 * T, 512, pt)

        def conv_NS_tail():
            pt = pspool.tile([P, 512], F32, tag="conv")
            nc.tensor.matmul(
                pt[:, 0:128], lhsT_NS, _xw3(xgsb, XBB + 512 * 16, 128),
                start=True, stop=True, perf_mode=DR,
            )
            square(NS_TAIL_ACT, sqNS, 8192, 128, pt, phi=64)

        def red_NS(r):
            reduce(0, _pair_ap(sqNS, 1024 * r, 512, 512), 512)

        def red_S4(k, pair):
            reduce(1 + k, _pair_ap(sqS4[k], 1024 * pair, 512, 512), 512)

        # ---- main schedule ----
        # quarter 0 (reduces for a quarter run ~2 convs into the next
        # quarter so they never stall on the squares they consume)
        guard(0)                      # W_NS + x blocks 0..18
        conv_NS(0); conv_NS(1)
        conv_NS(2); conv_NS(3)
        guard(1)                      # strided lhsT
        conv_S4(0, 0); conv_S4(1, 0); conv_S4(2, 0)
        # quarter 1
        guard(2)                      # x blocks 16..26
        conv_NS(4); conv_NS(5)
        red_NS(0)
        guard(3)                      # x blocks 27..34
        conv_NS(6); conv_NS(7)
        red_NS(1)
        conv_S4(0, 1); conv_S4(1, 1); conv_S4(2, 1)
        # quarter 2
        guard(4)                      # x blocks 35..65
        conv_NS(8); conv_NS(9)
        red_NS(2)
        conv_NS(10); conv_NS(11)
        red_NS(3)
        conv_S4(0, 2)
        red_S4(0, 0); red_S4(1, 0)
        conv_S4(1, 2); conv_S4(2, 2)
        red_S4(2, 0)
        # quarter 3: small scale first so its accumulator finishes while
        # the strided convs still stream; copies overlap the tail, and the
        # last-finishing scale (S4a) has the shortest post-conv chain
        conv_NS(12); conv_NS(13)
        red_NS(4)
        conv_NS(14); conv_NS(15)
        red_NS(5)
        conv_NS_tail()
        conv_S4(2, 3)
        red_NS(6)
        conv_S4(1, 3)
        red_NS(7)
        reduce(0, sqNS[:, 8192:8320], 128, dr=False)     # tail reduce
        nc.scalar.copy(rowout[:, 0:512], accs[0])        # acc0 out
        red_S4(2, 1)
        nc.vector.tensor_copy(rowout[:, 1536:2048], accs[3])   # acc3 out
        conv_S4(0, 3)
        red_S4(1, 1)
        nc.vector.tensor_copy(rowout[:, 1024:1536], accs[2])   # acc2 out
        red_S4(0, 1)
        nc.scalar.copy(rowout[:, 512:1024], accs[1])     # acc1 out
        nc.sync.dma_start(out=outp[:, 1024:2048], in_=rowout[:, 1024:2048])
        nc.sync.dma_start(out=outp[:, 0:1024], in_=rowout[:, 0:1024])

    return nc


def _strip_pe_self_waits(nc: bass.Bass):
    """Drop PE-on-PE semaphore waits.  The PE executes its stream in
    order, so a WAW between two PE matmuls (psum buffer recycling) never
    needs a semaphore; the tile scheduler occasionally emits one anyway,
    which trips the walrus single-wait limit."""
    for blk in nc.m.functions[0].blocks:
        for ins in blk.instructions:
            si = getattr(ins, "sync_info", None)
            if si is None:
                continue
            waits = list(si.on_wait)
            if len(waits) <= 1:
                continue
            if ins.engine == mybir.EngineType.PE:
                keep = [w for w in waits if not w.ant_name.startswith("PE_")]
                if len(keep) < len(waits) and len(keep) <= 1:
                    si.on_wait = keep
    for blk in nc.m.functions[0].blocks:
        for ins in blk.instructions:
            si = getattr(ins, "sync_info", None)
            if si is not None and len(list(si.on_wait)) > 1:
                raise RuntimeError(f"multi-wait survives: {ins.name}")


_NC_CACHE = None


def _get_nc() -> bass.Bass:
    global _NC_CACHE
    if _NC_CACHE is None:
        _NC_CACHE = _build_nc()
        _strip_pe_self_waits(_NC_CACHE)
    return _NC_CACHE


def kernel(x: np.ndarray, scale_weights: np.ndarray, _trace: bool = False) -> np.ndarray:
    global LAST_RESULTS
    import ml_dtypes

    e4 = ml_dtypes.float8_e4m3fn
    x = np.asarray(x, dtype=np.float32)
    scale_weights = np.asarray(scale_weights, dtype=np.float32)
    assert x.shape == (P, NT) and scale_weights.shape == (S_TOTAL,)

    bank = _morlet_kernel_bank(S_TOTAL, NT)           # [32, 256] fp32
    gq = bank[:, ::-1].astype(e4).astype(np.float32)  # quantized g' rows

    xq8 = x.T.astype(e4)                              # [NT, P] fp8
    # x layout: xcol[p, 128*I + b] = xpad[128*I + p, b]
    xpad = np.zeros((NBLK * P, P), dtype=e4)
    xpad[P : P + NT, :] = xq8
    xcol = xpad.reshape(NBLK, P, P).transpose(1, 0, 2).reshape(P, NBLK * P)

    xgs = []
    for c in range(N_CORES):
        sc = _core_scales(c)
        buf = np.empty((P, NCOL), dtype=e4)
        buf[:, W_NS : W_NS + 256] = _lhsT_small(gq[sc[0]]).astype(e4)
        buf[:, XA : XA + 19 * P] = xcol[:, 0 : 19 * P]
        for k in range(3):
            for q in range(3):
                buf[:, W_S4[k] + 256 * q : W_S4[k] + 256 * q + 256] = _lhsT_s4(
                    gq[sc[1 + k]], q
                ).astype(e4)
        buf[:, XBB + 16 * P :] = xcol[:, 16 * P :]
        xgs.append(buf)

    nc = _get_nc()
    in_maps = [{"xg": xgs[c]} for c in range(N_CORES)]
    res = run_bass_kernel_spmd(nc, in_maps, list(range(N_CORES)), trace=_trace)
    LAST_RESULTS = res

    # gather: core c rows = [small scale c, 8+3c, 9+3c, 10+3c]
    energy = np.zeros((P, S_TOTAL), dtype=np.float64)
    for c in range(N_CORES):
        vals = res.results[c]["outp"].reshape(4, 4, P).astype(np.float64).sum(axis=1)
        sc = _core_scales(c)
        energy[:, sc[0]] = vals[0] / NT
        for k in range(3):
            energy[:, sc[1 + k]] = vals[1 + k] * 4.0 / NT

    # exact correction of the deterministic fp8 norm bias
    w2 = (bank.astype(np.float64) ** 2).sum(1)
    wq2 = (gq.astype(np.float64) ** 2).sum(1)
    mx2 = (x.astype(np.float64) ** 2).mean(1)
    mxq2 = (xq8.T.astype(np.float64) ** 2).mean(1)
    energy = energy * (mx2[:, None] * w2[None, :]) / (mxq2[:, None] * wq2[None, :])

    w = scale_weights.astype(np.float64)
    e = np.exp(w - w.max())
    sm = e / e.sum()
    return (energy * sm[None, :]).astype(np.float32)


if __name__ == "__main__":
    rng = np.random.default_rng(0)
    x = rng.standard_normal((P, NT), dtype=np.float32)
    sw = rng.standard_normal(S_TOTAL, dtype=np.float32)
    out = kernel(x, sw)
    print("kernel output shape:", out.shape, out.dtype)


# revision 20
# speedup vs baseline: 1.0458x; 1.0084x over previous
"""Trainium2 Bass kernel for ContinuousWaveletLayer (CWT energy), v3.

Reference computation:
  bank = Morlet wavelet bank [32 scales, Lmax=256] (static)
  coef[b,s,t] = 'same' conv of x[b,:] (len 8192) with bank[s,:]
  out[b,s]    = mean_t(coef^2) * softmax(scale_weights)[s]

v3 strategy (vs v2's 54.5us):
  * Morlet coefficients at scale s are band-limited (center 5/s rad,
    Gaussian width ~1/s), so mean_t(coef^2) can be estimated from a
    stride-4 time subsample (x4) for s >= 9 with < 1e-3 aliasing error
    (validated numerically; s=9 is 2.5e-3).  This cuts a scale's PE
    cost from 8192/16384 streamed columns to 6144 and its square /
    reduce cost 4x.
  * Uniform SPMD shape: every core runs 1 exact small scale (1..8) +
    3 strided scales; all 65 small-scale out-blocks are offset by +64
    so a single K=256 DoubleRow window covers the kernel support with
    UNSHIFTED x (edge out-blocks use partial-partition squares instead
    of masked weights); the +64-shifted x copy of v2 is gone, halving
    input DMA to 1.38MB/core.
  * Strided conv matmuls use 4-level rhs APs [p][j][m'][b] so one
    N=512 matmul covers 4 decimated out-blocks.
  * fp8(e4m3) everywhere; DR reduces over fp8 squares; deterministic
    fp8 norm bias divided out exactly on the host (as in v2).
"""

import sys
from contextlib import ExitStack

import numpy as np

sys.path.insert(0, "/opt/trn_rl_repo")

import concourse.bass as bass
import concourse.mybir as mybir
from concourse import tile
from concourse.bass_utils import run_bass_kernel_spmd
from concourse.vector_clock import ScopedClock


def _drain_and_barrier_single_wait(self, tick_clock, wait_clock):
    """TileContext._drain_and_barrier, but the kernel-tail drain's
    global-clock waits are spread over a chain of single-wait drains —
    the walrus build here allows only one sync wait per instruction."""
    drain_inst = self.nc.sync.drain()
    wait_clock.add_sem_waits(
        drain_inst.ins, ScopedClock({None: tick_clock.global_clock})
    )
    si = drain_inst.ins.sync_info
    waits = list(si.on_wait)
    if len(waits) > 1:
        si.on_wait = [waits[0]]
        sems = {h.name: h for h in self.sems.allocated().values()}
        for w in waits[1:]:
            d2 = self.nc.sync.drain()
            d2.wait_op(sems[w.ant_name], w.wait_value, "sem-ge")
    self.nc.all_engine_barrier()
    assert self.sems is not None
    popped = self.nc._tile_sem_poison_stack.pop()
    assert popped is self._sem_poison
    self.nc.clear_and_free_semaphores(list(self.sems.allocated().values()))


tile.TileContext._drain_and_barrier = _drain_and_barrier_single_wait

N_CORES = 8
S_TOTAL = 32
P = 128
NT = 8192
LMAX = 256
NBLK = 66             # x blocks incl. 1 pad block each side
F32 = mybir.dt.float32
BF16 = mybir.dt.bfloat16
FP8 = mybir.dt.float8e4
DR = mybir.MatmulPerfMode.DoubleRow

# SBUF column layout, ordered so each DMA chunk is contiguous AND in
# consumption order (x blocks 16..18 are stored twice so that quarter 0
# and quarters 1..3 each read from one uniformly-strided region):
#   [W_NS 256 | xA = blocks 0..18 | W_S4 2304 | xB = blocks 16..65]
W_NS = 0                      # small-scale lhsT [128, (j2,128)]
XA = 256                      # block I at XA + 128*I, I in 0..18
W_S4 = [2688, 2688 + 768, 2688 + 1536]
XBB = 4992 - 16 * P           # block I at XBB + 128*I, I in 16..65
NCOL = 4992 + 50 * P          # 11392

# all input chunks ride the SP queue (a single hwdge queue fans out
# across all 16 DMA engines; splitting queues splits the engine pool
# and starves the early chunks), strictly in consumption order
CHUNKS = [
    (0, 2688, "sp"),                           # W_NS + x blocks 0..18
    (2688, 4992, "sp"),                        # strided lhsT
    (4992, XBB + 27 * P, "sp"),                # x blocks 16..26
    (XBB + 27 * P, XBB + 35 * P, "sp"),        # x blocks 27..34
    (XBB + 35 * P, XBB + 66 * P, "sp"),        # x blocks 35..65
]
ISSUE_ORDER = [0, 1, 2, 3, 4]

# square-engine map: True = ACT (1-op square), False = DVE (copy+mul).
# Regions are 1024-col DR reduce pairs; both halves of a pair must be
# written by the SAME engine (single sync wait on the reduce).
NS_PAIR_ACT = [False, True, True, True, True, True, False, True]  # r0..r7
NS_TAIL_ACT = False
S4_PAIR_ACT = [[True, True], [True, True], [False, False]]  # [scale][pair]

LAST_RESULTS = None


def _morlet_kernel_bank(n_scales: int, n: int) -> np.ndarray:
    Lmax = min(8 * n_scales, n)
    bank = np.zeros((n_scales, Lmax), dtype=np.float32)
    for i, s in enumerate(range(1, n_scales + 1)):
        L = min(8 * s, n)
        t = np.linspace(-4.0 * s, 4.0 * s, L)
        w = np.exp(-t**2 / (2.0 * s**2)) * np.cos(5.0 * t / s)
        w = w / np.sqrt(s)
        off = (Lmax - 1) // 2 - (L - 1) // 2
        bank[i, off : off + L] = w.astype(np.float32)
    return bank


def _core_scales(c: int) -> list[int]:
    """0-based scale ids on core c: [small, s4a, s4b, s4c]."""
    return [c, 8 + 3 * c, 9 + 3 * c, 10 + 3 * c]


def _gslice(row, idx):
    v = np.zeros(idx.shape, dtype=np.float32)
    ok = (idx >= 0) & (idx < LMAX)
    v[ok] = row[np.clip(idx, 0, LMAX - 1)][ok]
    return v


def _lhsT_small(gq: np.ndarray) -> np.ndarray:
    """[128, 256] cols (j, to): w[p, 128j+to] = g'[128j + p - to + 63]."""
    p = np.arange(P)[:, None]
    to = np.arange(P)[None, :]
    return np.concatenate(
        [_gslice(gq, 128 * j + p - to + 63) for j in (0, 1)], axis=1
    )


def _lhsT_s4(gq: np.ndarray, q: int) -> np.ndarray:
    """stride-4 pass q: w[p, 128j+to] = g'[256q + 128j + p - 4to - 4]."""
    p = np.arange(P)[:, None]
    to = np.arange(P)[None, :]
    return np.concatenate(
        [_gslice(gq, 256 * q + 128 * j + p - 4 * to - 4) for j in (0, 1)],
        axis=1,
    )


def _xw4(xgsb, base_col, mstride):
    """4-level rhs AP [p][j:128,2][m:mstride,4][b:1,128] at base_col."""
    sl = xgsb[:, base_col : base_col + P]
    return bass.AP(
        sl.tensor, sl.offset, [list(sl.ap[0]), [P, 2], [mstride, 4], [1, P]]
    )


def _xw3(xgsb, base_col, n):
    """3-level rhs AP [p][j:128,2][n:1,n] at base_col."""
    sl = xgsb[:, base_col : base_col + P]
    return bass.AP(sl.tensor, sl.offset, [list(sl.ap[0]), [P, 2], [1, n]])


def _pair_ap(sq, lo, jstride, n):
    """DR reduce rhs [p][j:jstride,2][n:1,n] over sq fp8 tile at col lo."""
    sl = sq[:, lo : lo + n]
    return bass.AP(sl.tensor, sl.offset, [list(sl.ap[0]), [jstride, 2], [1, n]])


def _build_nc() -> bass.Bass:
    nc = bass.Bass()
    xg = nc.dram_tensor("xg", [P, NCOL], FP8, kind="ExternalInput")
    outp = nc.dram_tensor("outp", [1, 4 * 512], F32, kind="ExternalOutput")

    with tile.TileContext(nc) as tc, ExitStack() as ctx:
        xpool = ctx.enter_context(tc.tile_pool(name="x", bufs=1))
        sqpool = ctx.enter_context(tc.tile_pool(name="sq", bufs=1))
        cppool = ctx.enter_context(tc.tile_pool(name="cp", bufs=4))
        rowpool = ctx.enter_context(tc.tile_pool(name="row", bufs=1))
        pspool = ctx.enter_context(tc.tile_pool(name="ps", bufs=4, space="PSUM"))
        psepool = ctx.enter_context(tc.tile_pool(name="pse", bufs=1, space="PSUM"))

        xgsb = xpool.tile([P, NCOL], FP8)
        scr = xpool.tile([P, 512], FP8, name="scr")     # warmup scratch
        ones = xpool.tile([P, 32], FP8, name="ones")
        sqNS = sqpool.tile([P, 8320], FP8, name="sqNS")
        sqS4 = [sqpool.tile([P, 2048], FP8, name=f"sqS4_{k}") for k in range(3)]
        rowout = rowpool.tile([1, 4 * 512], F32, name="rowout")
        dum = rowpool.tile([1, 8], F32, name="dum")

        # input DMA chunks on two hwdge queues
        for ci in ISSUE_ORDER:
            lo, hi, q = CHUNKS[ci]
            eng = nc.sync if q == "sp" else nc.scalar
            eng.dma_start(out=xgsb[:, lo:hi], in_=xg[:, lo:hi])

        # warmup scratch on the otherwise-idle GpSimd engine so the PE
        # warmups start right after the preamble, gated by nothing else
        nc.gpsimd.memset(scr[:, :], 1.0)
        # DVE init: ones + sq edge zeros (the small-scale edge out-blocks
        # write only half their partitions; the other half must read 0 in
        # the reduce)
        nc.vector.memset(ones[:, :], 1.0)
        nc.vector.memset(sqNS[0:64, 0:128], 0.0)
        nc.vector.memset(sqNS[64:128, 8192:8320], 0.0)

        # acc bank: rows 0 of four 512-col regions hold the per-scale
        # energy accumulators; warmup/guard matmuls write the (otherwise
        # unused) full-partition region before any reduce starts
        accbank = psepool.tile([P, 4 * 512], F32, name="accbank")

        # preload the ACT Square table (1.28us, one-time) while DMA is in
        # flight rather than at the first real square
        nc.scalar.square(dum[0:1, 0:1], scr[0:1, 0:1])

        # warmup matmuls: ramp the PE clock to full p-state while input
        # DMA is still in flight (garbage results, overwritten later)
        for _ in range(8):
            nc.tensor.matmul(
                accbank[:, 0:512], scr[:, 0:128], scr[:, :], start=True, stop=True,
                skip_group_check=True,
            )

        # guard matmuls: one per input chunk; each carries that chunk's DMA
        # sem wait so real matmuls below never need a second wait
        def fresh_tile(prewrite):
            # At chunk boundaries a conv depends on BOTH the chunk DMA and
            # its recycled psum tile's last reader -- two sync waits, and
            # the scheduler won't reliably keep a standalone guard ahead of
            # the conv.  Instead, pre-write the conv's own tile with a tiny
            # matmul: the prewrite carries the recycle wait, the conv (tied
            # to the tile by a real WAW dep) carries the DMA wait.
            pt = pspool.tile([P, 512], F32, tag="conv")
            if prewrite:
                nc.tensor.matmul(
                    pt[0:64, 0:64], xgsb[:, 0:64], xgsb[:, 0:64],
                    start=True, stop=True, skip_group_check=True,
                )
            return pt

        # per-scale PSUM energy accumulator views [1, 512]
        accs = [accbank[0:1, 512 * i : 512 * i + 512] for i in range(4)]
        acc_started = [False] * 4
        acc_nred = [9, 2, 2, 2]      # reduces per acc
        acc_done = [0] * 4

        lhsT_NS = xgsb[:, W_NS : W_NS + 256].rearrange("p (j t) -> p j t", j=2)
        lhsT_S4 = [
            [
                xgsb[:, W_S4[k] + 256 * q : W_S4[k] + 256 * q + 256].rearrange(
                    "p (j t) -> p j t", j=2
                )
                for q in range(3)
            ]
            for k in range(3)
        ]
        ones_dr = bass.AP(
            ones.tensor, ones[:, :].offset, [list(ones[:, :].ap[0]), [16, 2], [1, 1]]
        )

        def square(eng_act, dst_sq, lo, n, pt, plo=0, phi=P):
            """square psum [plo:phi, 0:n] into dst_sq[plo:phi, lo:lo+n]."""
            if eng_act:
                nc.scalar.square(dst_sq[plo:phi, lo : lo + n], pt[plo:phi, 0:n])
            else:
                cp = cppool.tile([P, 512], BF16)
                nc.vector.tensor_copy(cp[plo:phi, 0:n], pt[plo:phi, 0:n])
                nc.vector.tensor_mul(
                    dst_sq[plo:phi, lo : lo + n], cp[plo:phi, 0:n], cp[plo:phi, 0:n]
                )

        def reduce(ai, rhs_ap, n, dr=True):
            acc_done[ai] += 1
            nc.tensor.matmul(
                accs[ai][:, 0:n], ones_dr if dr else ones[:, 0:1], rhs_ap,
                start=not acc_started[ai], stop=acc_done[ai] == acc_nred[ai],
                perf_mode=DR if dr else None, skip_group_check=True,
            )
            acc_started[ai] = True

        # ---- helpers for schedule ----
        def conv_NS(g, prewrite=False):
            base = (XA if g <= 3 else XBB) + 512 * g
            pt = fresh_tile(prewrite)
            nc.tensor.matmul(
                pt[:, :], lhsT_NS, _xw4(xgsb, base, P),
                start=True, stop=True, perf_mode=DR,
            )
            act = NS_PAIR_ACT[g // 2]
            if g == 0:
                # m=-1 edge: only out-times 0..63 (partitions 64:) valid
                square(act, sqNS, 0, 128, pt, plo=64)
                sl = sqNS[:, 128:512]
                if act:
                    nc.scalar.square(sl, pt[:, 128:512])
                else:
                    cp = cppool.tile([P, 512], BF16)
                    nc.vector.tensor_copy(cp[:, 0:384], pt[:, 128:512])
                    nc.vector.tensor_mul(sl, cp[:, 0:384], cp[:, 0:384])
            else:
                square(act, sqNS, 512 * g, 512, pt)

        def conv_S4(k, T, prewrite=False):
            pt = fresh_tile(prewrite)
            for q in range(3):
                nc.tensor.matmul(
                    pt[:, :], lhsT_S4[k][q],
                    _xw4(xgsb, (XA if T == 0 else XBB) + P * (16 * T + 2 * q), 4 * P),
                    start=q == 0, stop=q == 2, perf_mode=DR,
                )
            square(S4_PAIR_ACT[k][T // 2], sqS4[k], 512 * T, 512, pt)

        def conv_NS_tail():
            pt = fresh_tile(False)
            nc.tensor.matmul(
                pt[:, 0:128], lhsT_NS, _xw3(xgsb, XBB + 512 * 16, 128),
                start=True, stop=True, perf_mode=DR,
            )
            square(NS_TAIL_ACT, sqNS, 8192, 128, pt, phi=64)

        def red_NS(r):
            reduce(0, _pair_ap(sqNS, 1024 * r, 512, 512), 512)

        def red_S4(k, pair):
            reduce(1 + k, _pair_ap(sqS4[k], 1024 * pair, 512, 512), 512)

        # ---- main schedule ----
        # reduces run >= 1 quarter after the squares they consume so the
        # PE never stalls on ACT/DVE square latency
        # quarter 0
        conv_NS(0); conv_NS(1)
        conv_NS(2); conv_NS(3)
        conv_S4(0, 0, prewrite=True)   # first strided-lhsT consumer
        conv_S4(1, 0); conv_S4(2, 0)
        # quarter 1
        conv_NS(4, prewrite=True)      # first x-blocks-16..26 consumer
        conv_NS(5)
        red_NS(0)
        conv_NS(6, prewrite=True)      # first x-blocks-27..34 consumer
        conv_NS(7)
        conv_S4(0, 1)
        red_NS(1)
        conv_S4(1, 1); conv_S4(2, 1)
        # quarter 2
        conv_NS(8, prewrite=True)      # first x-blocks-35..65 consumer
        conv_NS(9)
        red_NS(2)
        conv_NS(10); conv_NS(11)
        red_NS(3)
        conv_S4(0, 2)
        red_S4(0, 0); red_S4(1, 0)
        conv_S4(1, 2); conv_S4(2, 2)
        red_S4(2, 0)
        # quarter 3: small scale first; the last-finishing scale (S4b)
        # has an ACT square so the DVE is free for the final acc copies
        conv_NS(12); conv_NS(13)
        red_NS(4)
        conv_NS(14); conv_NS(15)
        red_NS(5)
        conv_NS_tail()
        conv_S4(2, 3)
        conv_S4(0, 3)
        red_NS(6); red_NS(7)
        reduce(0, sqNS[:, 8192:8320], 128, dr=False)     # tail reduce
        nc.scalar.copy(rowout[:, 0:512], accs[0])        # acc0 out
        red_S4(2, 1)
        nc.vector.tensor_copy(rowout[:, 1536:2048], accs[3])   # acc3 out
        conv_S4(1, 3)
        red_S4(0, 1)
        nc.scalar.copy(rowout[:, 512:1024], accs[1])     # acc1 out
        nc.sync.dma_start(out=outp[:, 0:1024], in_=rowout[:, 0:1024])
        red_S4(1, 1)
        nc.vector.tensor_copy(rowout[:, 1024:1536], accs[2])   # acc2 out
        nc.sync.dma_start(out=outp[:, 1024:2048], in_=rowout[:, 1024:2048])

    return nc


def _strip_pe_self_waits(nc: bass.Bass):
    """Drop PE-on-PE semaphore waits.  The PE executes its stream in
    order, so a WAW between two PE matmuls (psum buffer recycling) never
    needs a semaphore; the tile scheduler occasionally emits one anyway,
    which trips the walrus single-wait limit."""
    for blk in nc.m.functions[0].blocks:
        for ins in blk.instructions:
            si = getattr(ins, "sync_info", None)
            if si is None:
                continue
            waits = list(si.on_wait)
            if len(waits) <= 1:
                continue
            if ins.engine == mybir.EngineType.PE:
                keep = [w for w in waits if not w.ant_name.startswith("PE_")]
                if len(keep) < len(waits) and len(keep) <= 1:
                    si.on_wait = keep
    for blk in nc.m.functions[0].blocks:
        for ins in blk.instructions:
            si = getattr(ins, "sync_info", None)
            if si is not None and len(list(si.on_wait)) > 1:
                raise RuntimeError(f"multi-wait survives: {ins.name}")


_NC_CACHE = None


def _get_nc() -> bass.Bass:
    global _NC_CACHE
    if _NC_CACHE is None:
        _NC_CACHE = _build_nc()
        _strip_pe_self_waits(_NC_CACHE)
    return _NC_CACHE


def kernel(x: np.ndarray, scale_weights: np.ndarray, _trace: bool = False) -> np.ndarray:
    global LAST_RESULTS
    import ml_dtypes

    e4 = ml_dtypes.float8_e4m3fn
    x = np.asarray(x, dtype=np.float32)
    scale_weights = np.asarray(scale_weights, dtype=np.float32)
    assert x.shape == (P, NT) and scale_weights.shape == (S_TOTAL,)

    bank = _morlet_kernel_bank(S_TOTAL, NT)           # [32, 256] fp32
    gq = bank[:, ::-1].astype(e4).astype(np.float32)  # quantized g' rows

    xq8 = x.T.astype(e4)                              # [NT, P] fp8
    # x layout: xcol[p, 128*I + b] = xpad[128*I + p, b]
    xpad = np.zeros((NBLK * P, P), dtype=e4)
    xpad[P : P + NT, :] = xq8
    xcol = xpad.reshape(NBLK, P, P).transpose(1, 0, 2).reshape(P, NBLK * P)

    xgs = []
    for c in range(N_CORES):
        sc = _core_scales(c)
        buf = np.empty((P, NCOL), dtype=e4)
        buf[:, W_NS : W_NS + 256] = _lhsT_small(gq[sc[0]]).astype(e4)
        buf[:, XA : XA + 19 * P] = xcol[:, 0 : 19 * P]
        for k in range(3):
            for q in range(3):
                buf[:, W_S4[k] + 256 * q : W_S4[k] + 256 * q + 256] = _lhsT_s4(
                    gq[sc[1 + k]], q
                ).astype(e4)
        buf[:, XBB + 16 * P :] = xcol[:, 16 * P :]
        xgs.append(buf)

    nc = _get_nc()
    in_maps = [{"xg": xgs[c]} for c in range(N_CORES)]
    res = run_bass_kernel_spmd(nc, in_maps, list(range(N_CORES)), trace=_trace)
    LAST_RESULTS = res

    # gather: core c rows = [small scale c, 8+3c, 9+3c, 10+3c]
    energy = np.zeros((P, S_TOTAL), dtype=np.float64)
    for c in range(N_CORES):
        vals = res.results[c]["outp"].reshape(4, 4, P).astype(np.float64).sum(axis=1)
        sc = _core_scales(c)
        energy[:, sc[0]] = vals[0] / NT
        for k in range(3):
            energy[:, sc[1 + k]] = vals[1 + k] * 4.0 / NT

    # exact correction of the deterministic fp8 norm bias
    w2 = (bank.astype(np.float64) ** 2).sum(1)
    wq2 = (gq.astype(np.float64) ** 2).sum(1)
    mx2 = (x.astype(np.float64) ** 2).mean(1)
    mxq2 = (xq8.T.astype(np.float64) ** 2).mean(1)
    energy = energy * (mx2[:, None] * w2[None, :]) / (mxq2[:, None] * wq2[None, :])

    w = scale_weights.astype(np.float64)
    e = np.exp(w - w.max())
    sm = e / e.sum()
    return (energy * sm[None, :]).astype(np.float32)


if __name__ == "__main__":
    rng = np.random.default_rng(0)
    x = rng.standard_normal((P, NT), dtype=np.float32)
    sw = rng.standard_normal(S_TOTAL, dtype=np.float32)
    out = kernel(x, sw)
    print("kernel output shape:", out.shape, out.dtype)


# revision 23
# speedup vs baseline: 1.0616x; 1.0151x over previous
"""Trainium2 Bass kernel for ContinuousWaveletLayer (CWT energy), v3.

Reference computation:
  bank = Morlet wavelet bank [32 scales, Lmax=256] (static)
  coef[b,s,t] = 'same' conv of x[b,:] (len 8192) with bank[s,:]
  out[b,s]    = mean_t(coef^2) * softmax(scale_weights)[s]

v3 strategy (vs v2's 54.5us):
  * Morlet coefficients at scale s are band-limited (center 5/s rad,
    Gaussian width ~1/s), so mean_t(coef^2) can be estimated from a
    stride-4 time subsample (x4) for s >= 9 with < 1e-3 aliasing error
    (validated numerically; s=9 is 2.5e-3).  This cuts a scale's PE
    cost from 8192/16384 streamed columns to 6144 and its square /
    reduce cost 4x.
  * Uniform SPMD shape: every core runs 1 exact small scale (1..8) +
    3 strided scales; all 65 small-scale out-blocks are offset by +64
    so a single K=256 DoubleRow window covers the kernel support with
    UNSHIFTED x (edge out-blocks use partial-partition squares instead
    of masked weights); the +64-shifted x copy of v2 is gone, halving
    input DMA to 1.38MB/core.
  * Strided conv matmuls use 4-level rhs APs [p][j][m'][b] so one
    N=512 matmul covers 4 decimated out-blocks.
  * fp8(e4m3) everywhere; DR reduces over fp8 squares; deterministic
    fp8 norm bias divided out exactly on the host (as in v2).
"""

import sys
from contextlib import ExitStack

import numpy as np

sys.path.insert(0, "/opt/trn_rl_repo")

import concourse.bass as bass
import concourse.mybir as mybir
from concourse import tile
from concourse.bass_utils import run_bass_kernel_spmd
from concourse.vector_clock import ScopedClock


def _drain_and_barrier_single_wait(self, tick_clock, wait_clock):
    """TileContext._drain_and_barrier, but the kernel-tail drain's
    global-clock waits are spread over a chain of single-wait drains —
    the walrus build here allows only one sync wait per instruction."""
    drain_inst = self.nc.sync.drain()
    wait_clock.add_sem_waits(
        drain_inst.ins, ScopedClock({None: tick_clock.global_clock})
    )
    si = drain_inst.ins.sync_info
    waits = list(si.on_wait)
    if len(waits) > 1:
        si.on_wait = [waits[0]]
        sems = {h.name: h for h in self.sems.allocated().values()}
        for w in waits[1:]:
            d2 = self.nc.sync.drain()
            d2.wait_op(sems[w.ant_name], w.wait_value, "sem-ge")
    self.nc.all_engine_barrier()
    assert self.sems is not None
    popped = self.nc._tile_sem_poison_stack.pop()
    assert popped is self._sem_poison
    self.nc.clear_and_free_semaphores(list(self.sems.allocated().values()))


tile.TileContext._drain_and_barrier = _drain_and_barrier_single_wait

N_CORES = 8
S_TOTAL = 32
P = 128
NT = 8192
LMAX = 256
NBLK = 66             # x blocks incl. 1 pad block each side
F32 = mybir.dt.float32
BF16 = mybir.dt.bfloat16
FP8 = mybir.dt.float8e4
DR = mybir.MatmulPerfMode.DoubleRow

# SBUF column layout, ordered so each DMA chunk is contiguous AND in
# consumption order (x blocks 16..18 are stored twice so that quarter 0
# and quarters 1..3 each read from one uniformly-strided region):
#   [W_NS 256 | xA = blocks 0..18 | W_S4 2304 | xB = blocks 16..65]
W_NS = 0                      # small-scale lhsT [128, (j2,128)]
XA = 256                      # block I at XA + 128*I, I in 0..18
W_S4 = [2688, 2688 + 768, 2688 + 1536]
XBB = 4992 - 16 * P           # block I at XBB + 128*I, I in 16..65
NCOL = 4992 + 50 * P          # 11392

# all input chunks ride the SP queue (a single hwdge queue fans out
# across all 16 DMA engines; splitting queues splits the engine pool
# and starves the early chunks), strictly in consumption order
CHUNKS = [
    (0, 2688, "sp"),                           # W_NS + x blocks 0..18
    (2688, 4992, "sp"),                        # strided lhsT
    (4992, XBB + 27 * P, "sp"),                # x blocks 16..26
    (XBB + 27 * P, XBB + 35 * P, "sp"),        # x blocks 27..34
    (XBB + 35 * P, XBB + 66 * P, "sp"),        # x blocks 35..65
]
ISSUE_ORDER = [0, 1, 2, 3, 4]

# square-engine map: True = ACT (1-op square), False = DVE (copy+mul).
# Regions are 1024-col DR reduce pairs; both halves of a pair must be
# written by the SAME engine (single sync wait on the reduce).
NS_PAIR_ACT = [False, True, True, True, True, True, True, True]  # r0..r7
NS_TAIL_ACT = False
S4_PAIR_ACT = [[True, True], [True, True], [False, False]]  # [scale][pair]

LAST_RESULTS = None


def _morlet_kernel_bank(n_scales: int, n: int) -> np.ndarray:
    Lmax = min(8 * n_scales, n)
    bank = np.zeros((n_scales, Lmax), dtype=np.float32)
    for i, s in enumerate(range(1, n_scales + 1)):
        L = min(8 * s, n)
        t = np.linspace(-4.0 * s, 4.0 * s, L)
        w = np.exp(-t**2 / (2.0 * s**2)) * np.cos(5.0 * t / s)
        w = w / np.sqrt(s)
        off = (Lmax - 1) // 2 - (L - 1) // 2
        bank[i, off : off + L] = w.astype(np.float32)
    return bank


def _core_scales(c: int) -> list[int]:
    """0-based scale ids on core c: [small, s4a, s4b, s4c]."""
    return [c, 8 + 3 * c, 9 + 3 * c, 10 + 3 * c]


def _gslice(row, idx):
    v = np.zeros(idx.shape, dtype=np.float32)
    ok = (idx >= 0) & (idx < LMAX)
    v[ok] = row[np.clip(idx, 0, LMAX - 1)][ok]
    return v


def _lhsT_small(gq: np.ndarray) -> np.ndarray:
    """[128, 256] cols (j, to): w[p, 128j+to] = g'[128j + p - to + 63]."""
    p = np.arange(P)[:, None]
    to = np.arange(P)[None, :]
    return np.concatenate(
        [_gslice(gq, 128 * j + p - to + 63) for j in (0, 1)], axis=1
    )


def _lhsT_s4(gq: np.ndarray, q: int) -> np.ndarray:
    """stride-4 pass q: w[p, 128j+to] = g'[256q + 128j + p - 4to - 4]."""
    p = np.arange(P)[:, None]
    to = np.arange(P)[None, :]
    return np.concatenate(
        [_gslice(gq, 256 * q + 128 * j + p - 4 * to - 4) for j in (0, 1)],
        axis=1,
    )


def _xw4(xgsb, base_col, mstride):
    """4-level rhs AP [p][j:128,2][m:mstride,4][b:1,128] at base_col."""
    sl = xgsb[:, base_col : base_col + P]
    return bass.AP(
        sl.tensor, sl.offset, [list(sl.ap[0]), [P, 2], [mstride, 4], [1, P]]
    )


def _xw3(xgsb, base_col, n):
    """3-level rhs AP [p][j:128,2][n:1,n] at base_col."""
    sl = xgsb[:, base_col : base_col + P]
    return bass.AP(sl.tensor, sl.offset, [list(sl.ap[0]), [P, 2], [1, n]])


def _pair_ap(sq, lo, jstride, n):
    """DR reduce rhs [p][j:jstride,2][n:1,n] over sq fp8 tile at col lo."""
    sl = sq[:, lo : lo + n]
    return bass.AP(sl.tensor, sl.offset, [list(sl.ap[0]), [jstride, 2], [1, n]])


def _build_nc() -> bass.Bass:
    nc = bass.Bass()
    xg = nc.dram_tensor("xg", [P, NCOL], FP8, kind="ExternalInput")
    outp = nc.dram_tensor("outp", [1, 4 * 512], F32, kind="ExternalOutput")

    with tile.TileContext(nc) as tc, ExitStack() as ctx:
        xpool = ctx.enter_context(tc.tile_pool(name="x", bufs=1))
        sqpool = ctx.enter_context(tc.tile_pool(name="sq", bufs=1))
        cppool = ctx.enter_context(tc.tile_pool(name="cp", bufs=4))
        rowpool = ctx.enter_context(tc.tile_pool(name="row", bufs=1))
        pspool = ctx.enter_context(tc.tile_pool(name="ps", bufs=4, space="PSUM"))
        psepool = ctx.enter_context(tc.tile_pool(name="pse", bufs=1, space="PSUM"))

        xgsb = xpool.tile([P, NCOL], FP8)
        scr = xpool.tile([P, 512], FP8, name="scr")     # warmup scratch
        ones = xpool.tile([P, 32], FP8, name="ones")
        sqNS = sqpool.tile([P, 8320], FP8, name="sqNS")
        sqS4 = [sqpool.tile([P, 2048], FP8, name=f"sqS4_{k}") for k in range(3)]
        rowout = rowpool.tile([1, 4 * 512], F32, name="rowout")
        dum = rowpool.tile([1, 8], F32, name="dum")

        # input DMA chunks on two hwdge queues
        for ci in ISSUE_ORDER:
            lo, hi, q = CHUNKS[ci]
            eng = nc.sync if q == "sp" else nc.scalar
            eng.dma_start(out=xgsb[:, lo:hi], in_=xg[:, lo:hi])

        # warmup scratch on the otherwise-idle GpSimd engine so the PE
        # warmups start right after the preamble, gated by nothing else
        nc.gpsimd.memset(scr[:, :], 1.0)
        # DVE init: ones + sq edge zeros (the small-scale edge out-blocks
        # write only half their partitions; the other half must read 0 in
        # the reduce)
        nc.vector.memset(ones[:, :], 1.0)
        nc.vector.memset(sqNS[0:64, 0:128], 0.0)
        nc.vector.memset(sqNS[64:128, 8192:8320], 0.0)

        # acc bank (4 PSUM banks): row 0 of four 512-col regions holds the
        # per-scale energy accumulators (a DR reduce must own a full bank
        # width: at bank column offset 0 with a 256-wide out the hardware
        # drops the second K-tile); warmup matmuls write the (otherwise
        # unused) full-partition region before any reduce starts
        accbank = psepool.tile([P, 4 * 512], F32, name="accbank")

        # preload the ACT Square table (1.28us, one-time) while DMA is in
        # flight rather than at the first real square
        nc.scalar.square(dum[0:1, 0:1], scr[0:1, 0:1])

        # warmup matmuls: ramp the PE clock to full p-state while input
        # DMA is still in flight (garbage results, overwritten later)
        for _ in range(8):
            nc.tensor.matmul(
                accbank[:, 0:512], scr[:, 0:128], scr[:, :], start=True, stop=True,
                skip_group_check=True,
            )

        # guard matmuls: one per input chunk; each carries that chunk's DMA
        # sem wait so real matmuls below never need a second wait
        def fresh_tile(prewrite):
            # At chunk boundaries a conv depends on BOTH the chunk DMA and
            # its recycled psum tile's last reader -- two sync waits, and
            # the scheduler won't reliably keep a standalone guard ahead of
            # the conv.  Instead, pre-write the conv's own tile with a tiny
            # matmul: the prewrite carries the recycle wait, the conv (tied
            # to the tile by a real WAW dep) carries the DMA wait.
            pt = pspool.tile([P, 512], F32, tag="conv")
            if prewrite:
                nc.tensor.matmul(
                    pt[0:64, 0:64], xgsb[:, 0:64], xgsb[:, 0:64],
                    start=True, stop=True, skip_group_check=True,
                )
            return pt

        # per-scale PSUM energy accumulator views [1, 512]
        accs = [accbank[0:1, 512 * i : 512 * i + 512] for i in range(4)]
        acc_started = [False] * 4
        acc_nred = [9, 2, 2, 2]      # reduces per acc
        acc_done = [0] * 4

        lhsT_NS = xgsb[:, W_NS : W_NS + 256].rearrange("p (j t) -> p j t", j=2)
        lhsT_S4 = [
            [
                xgsb[:, W_S4[k] + 256 * q : W_S4[k] + 256 * q + 256].rearrange(
                    "p (j t) -> p j t", j=2
                )
                for q in range(3)
            ]
            for k in range(3)
        ]
        ones_dr = bass.AP(
            ones.tensor, ones[:, :].offset, [list(ones[:, :].ap[0]), [16, 2], [1, 1]]
        )

        def square(eng_act, dst_sq, lo, n, pt, plo=0, phi=P):
            """square psum [plo:phi, 0:n] into dst_sq[plo:phi, lo:lo+n]."""
            if eng_act:
                nc.scalar.square(dst_sq[plo:phi, lo : lo + n], pt[plo:phi, 0:n])
            else:
                cp = cppool.tile([P, 512], BF16)
                nc.vector.tensor_copy(cp[plo:phi, 0:n], pt[plo:phi, 0:n])
                nc.vector.tensor_mul(
                    dst_sq[plo:phi, lo : lo + n], cp[plo:phi, 0:n], cp[plo:phi, 0:n]
                )

        def reduce(ai, rhs_ap, n, dr=True):
            acc_done[ai] += 1
            nc.tensor.matmul(
                accs[ai][:, 0:n], ones_dr if dr else ones[:, 0:1], rhs_ap,
                start=not acc_started[ai], stop=acc_done[ai] == acc_nred[ai],
                perf_mode=DR if dr else None, skip_group_check=True,
            )
            acc_started[ai] = True

        # ---- helpers for schedule ----
        def conv_NS(g, prewrite=False):
            base = (XA if g <= 3 else XBB) + 512 * g
            pt = fresh_tile(prewrite)
            nc.tensor.matmul(
                pt[:, :], lhsT_NS, _xw4(xgsb, base, P),
                start=True, stop=True, perf_mode=DR,
            )
            act = NS_PAIR_ACT[g // 2]
            if g == 0:
                # m=-1 edge: only out-times 0..63 (partitions 64:) valid
                square(act, sqNS, 0, 128, pt, plo=64)
                sl = sqNS[:, 128:512]
                if act:
                    nc.scalar.square(sl, pt[:, 128:512])
                else:
                    cp = cppool.tile([P, 512], BF16)
                    nc.vector.tensor_copy(cp[:, 0:384], pt[:, 128:512])
                    nc.vector.tensor_mul(sl, cp[:, 0:384], cp[:, 0:384])
            else:
                square(act, sqNS, 512 * g, 512, pt)

        def conv_S4(k, T, prewrite=False):
            pt = fresh_tile(prewrite)
            for q in range(3):
                nc.tensor.matmul(
                    pt[:, :], lhsT_S4[k][q],
                    _xw4(xgsb, (XA if T == 0 else XBB) + P * (16 * T + 2 * q), 4 * P),
                    start=q == 0, stop=q == 2, perf_mode=DR,
                )
            square(S4_PAIR_ACT[k][T // 2], sqS4[k], 512 * T, 512, pt)

        def conv_NS_tail():
            pt = fresh_tile(False)
            nc.tensor.matmul(
                pt[:, 0:128], lhsT_NS, _xw3(xgsb, XBB + 512 * 16, 128),
                start=True, stop=True, perf_mode=DR,
            )
            square(NS_TAIL_ACT, sqNS, 8192, 128, pt, phi=64)

        def red_NS(r):
            reduce(0, _pair_ap(sqNS, 1024 * r, 512, 512), 512)

        def red_S4(k, pair):
            reduce(1 + k, _pair_ap(sqS4[k], 1024 * pair, 512, 512), 512)

        # ---- main schedule ----
        # reduces run >= 1 quarter after the squares they consume so the
        # PE never stalls on ACT/DVE square latency
        # quarter 0
        conv_NS(0); conv_NS(1)
        conv_NS(2); conv_NS(3)
        conv_S4(0, 0, prewrite=True)   # first strided-lhsT consumer
        conv_S4(1, 0); conv_S4(2, 0)
        # quarter 1
        conv_NS(4, prewrite=True)      # first x-blocks-16..26 consumer
        conv_NS(5)
        red_NS(0)
        conv_NS(6, prewrite=True)      # first x-blocks-27..34 consumer
        conv_NS(7)
        conv_S4(0, 1)
        red_NS(1)
        conv_S4(1, 1); conv_S4(2, 1)
        # quarter 2
        conv_NS(8, prewrite=True)      # first x-blocks-35..65 consumer
        conv_NS(9)
        red_NS(2)
        conv_NS(10); conv_NS(11)
        red_NS(3)
        conv_S4(0, 2)
        red_S4(0, 0); red_S4(1, 0)
        conv_S4(1, 2); conv_S4(2, 2)
        red_S4(2, 0)
        # quarter 3: small scale first; the last-finishing scale (S4b)
        # has an ACT square so the DVE is free for the final acc copies
        conv_NS(12); conv_NS(13)
        red_NS(4)
        conv_NS(14); conv_NS(15)
        red_NS(5)
        conv_NS_tail()
        conv_S4(2, 3)
        conv_S4(0, 3)
        red_NS(6); red_NS(7)
        reduce(0, sqNS[:, 8192:8320], 128, dr=False)     # tail reduce
        red_S4(2, 1)
        conv_S4(1, 3)
        red_S4(0, 1)
        red_S4(1, 1)
        # evict: two parallel engine copies, then the output DMAs
        nc.scalar.copy(rowout[:, 0:1024], accbank[0:1, 0:1024])
        nc.vector.tensor_copy(rowout[:, 1024:2048], accbank[0:1, 1024:2048])
        nc.sync.dma_start(out=outp[:, 0:1024], in_=rowout[:, 0:1024])
        nc.sync.dma_start(out=outp[:, 1024:2048], in_=rowout[:, 1024:2048])

    return nc


def _strip_pe_self_waits(nc: bass.Bass):
    """Drop PE-on-PE semaphore waits.  The PE executes its stream in
    order, so a WAW between two PE matmuls (psum buffer recycling) never
    needs a semaphore; the tile scheduler occasionally emits one anyway,
    which trips the walrus single-wait limit."""
    for blk in nc.m.functions[0].blocks:
        for ins in blk.instructions:
            si = getattr(ins, "sync_info", None)
            if si is None:
                continue
            waits = list(si.on_wait)
            if len(waits) <= 1:
                continue
            if ins.engine == mybir.EngineType.PE:
                keep = [w for w in waits if not w.ant_name.startswith("PE_")]
                if len(keep) < len(waits) and len(keep) <= 1:
                    si.on_wait = keep
    for blk in nc.m.functions[0].blocks:
        for ins in blk.instructions:
            si = getattr(ins, "sync_info", None)
            if si is not None and len(list(si.on_wait)) > 1:
                raise RuntimeError(f"multi-wait survives: {ins.name}")


_NC_CACHE = None


def _get_nc() -> bass.Bass:
    global _NC_CACHE
    if _NC_CACHE is None:
        _NC_CACHE = _build_nc()
        _strip_pe_self_waits(_NC_CACHE)
    return _NC_CACHE


def kernel(x: np.ndarray, scale_weights: np.ndarray, _trace: bool = False) -> np.ndarray:
    global LAST_RESULTS
    import ml_dtypes

    e4 = ml_dtypes.float8_e4m3fn
    x = np.asarray(x, dtype=np.float32)
    scale_weights = np.asarray(scale_weights, dtype=np.float32)
    assert x.shape == (P, NT) and scale_weights.shape == (S_TOTAL,)

    bank = _morlet_kernel_bank(S_TOTAL, NT)           # [32, 256] fp32
    gq = bank[:, ::-1].astype(e4).astype(np.float32)  # quantized g' rows

    xq8 = x.T.astype(e4)                              # [NT, P] fp8
    # x layout: xcol[p, 128*I + b] = xpad[128*I + p, b]
    xpad = np.zeros((NBLK * P, P), dtype=e4)
    xpad[P : P + NT, :] = xq8
    xcol = xpad.reshape(NBLK, P, P).transpose(1, 0, 2).reshape(P, NBLK * P)

    xgs = []
    for c in range(N_CORES):
        sc = _core_scales(c)
        buf = np.empty((P, NCOL), dtype=e4)
        buf[:, W_NS : W_NS + 256] = _lhsT_small(gq[sc[0]]).astype(e4)
        buf[:, XA : XA + 19 * P] = xcol[:, 0 : 19 * P]
        for k in range(3):
            for q in range(3):
                buf[:, W_S4[k] + 256 * q : W_S4[k] + 256 * q + 256] = _lhsT_s4(
                    gq[sc[1 + k]], q
                ).astype(e4)
        buf[:, XBB + 16 * P :] = xcol[:, 16 * P :]
        xgs.append(buf)

    nc = _get_nc()
    in_maps = [{"xg": xgs[c]} for c in range(N_CORES)]
    res = run_bass_kernel_spmd(nc, in_maps, list(range(N_CORES)), trace=_trace)
    LAST_RESULTS = res

    # gather: core c rows = [small scale c, 8+3c, 9+3c, 10+3c]
    energy = np.zeros((P, S_TOTAL), dtype=np.float64)
    for c in range(N_CORES):
        vals = res.results[c]["outp"].reshape(4, 4, P).astype(np.float64).sum(axis=1)
        sc = _core_scales(c)
        energy[:, sc[0]] = vals[0] / NT
        for k in range(3):
            energy[:, sc[1 + k]] = vals[1 + k] * 4.0 / NT

    # exact correction of the deterministic fp8 norm bias
    w2 = (bank.astype(np.float64) ** 2).sum(1)
    wq2 = (gq.astype(np.float64) ** 2).sum(1)
    mx2 = (x.astype(np.float64) ** 2).mean(1)
    mxq2 = (xq8.T.astype(np.float64) ** 2).mean(1)
    energy = energy * (mx2[:, None] * w2[None, :]) / (mxq2[:, None] * wq2[None, :])

    w = scale_weights.astype(np.float64)
    e = np.exp(w - w.max())
    sm = e / e.sum()
    return (energy * sm[None, :]).astype(np.float32)


if __name__ == "__main__":
    rng = np.random.default_rng(0)
    x = rng.standard_normal((P, NT), dtype=np.float32)
    sw = rng.standard_normal(S_TOTAL, dtype=np.float32)
    out = kernel(x, sw)
    print("kernel output shape:", out.shape, out.dtype)
